# revision 55
# baseline (speedup 1.0000x reference)
"""Trainium2 Bass kernel for the BDH dense-transformer problem.

Sharding: data-parallel over B=8 across the 8 NeuronCores (one batch
element per core, no collectives). Each core runs the full 6-layer
network on its [T=2048, D=256] slice.

Per-core program. Matmul precision strategy:
  - the per-layer FLOP bulk (attention energy/a, MLP x/y/update) runs
    in float32r: 1 cyc/row on the PE when the output free dim is
    >= 256 (true for all matmuls here), vs 4 cyc/row for fp32 and
    3 cyc/row for the bf16x2 3-pass split scheme. No host splits and
    no DVE split work. All tensors feeding an f32r matmul are declared
    float32r so writes round appropriately (BIR verifier requirement).
  - precision recovery: the residual stream vN and the update
    accumulator updW stay full fp32 (vNr is a rounded F32R mirror used
    only as the attention a-matmul input), and the run-once embedding
    and readout matmuls are full fp32. Only per-layer matmul-input
    roundings remain.
Structure:
  - token embedding via one-hot matmul (iota + is_equal + PE)
  - v kept in both layouts: vT [D,T] (f32r) and vN [T,D] (fp32)
  - causal linear attention block-wise: energyT = qr@qr^T per
    [s128, t512] block (PSUM), bf16-mask multiply, then aN accumulated
    in PSUM over s-chunks
  - LayerNorms in natural layout with fused ACT Square/Identity
    (per-partition scale+bias + accum_out row sums)
  - MLP streamed over N in eighths (fp32 weights DMA'd per layer in
    host-pre-shuffled partition-contiguous layouts),
    relu(x)*relu(y) fused via scalar_tensor_tensor, update accumulated
    in PSUM then SBUF
  - PE 128x128 transposes maintain both v layouts
"""

import math

import numpy as np
import ml_dtypes

import concourse.bass as bass
import concourse.tile as tile
from concourse import bacc, mybir
from concourse import bass_utils

F32 = mybir.dt.float32
F32R = mybir.dt.float32r
BF16 = mybir.dt.bfloat16
I32 = mybir.dt.int32
ALU = mybir.AluOpType
ACTF = mybir.ActivationFunctionType
AXX = mybir.AxisListType.X

B, T, D, N, H, VOCAB, L = 8, 2048, 256, 8192, 4, 256, 6
EPS = 1e-5
TS = 512          # t-super width
NSUP = T // TS    # 4
NTB = T // 128    # 16
NQ = 8            # weight chunks along N
NCHQ = N // 128 // NQ  # 8 n-chunks per weight chunk


def build_nc(layers=L, stream_weights=True, attn=True, cphase=True):
    nc = bacc.Bacc("TRN2", target_bir_lowering=False, debug=False)

    idx_d = nc.dram_tensor("idxf", [1, T], F32R, kind="ExternalInput")
    wte_d = nc.dram_tensor("wte", [VOCAB, D], F32, kind="ExternalInput")
    wx_d = nc.dram_tensor("wx", [128, 2, N], F32R, kind="ExternalInput")
    wy_d = nc.dram_tensor("wy", [128, 2, N], F32R, kind="ExternalInput")
    enc_d = nc.dram_tensor("enc", [128, N // 128, D], F32R, kind="ExternalInput")
    ro_d = nc.dram_tensor("ro", [D, VOCAB], F32, kind="ExternalInput")
    cos_d = nc.dram_tensor("cosT", [128, T], F32, kind="ExternalInput")
    sin_d = nc.dram_tensor("sinT", [128, T], F32, kind="ExternalInput")
    mask_d = nc.dram_tensor("maskbig", [128, 1024], BF16, kind="ExternalInput")
    ident_d = nc.dram_tensor("identm", [128, 128], F32, kind="ExternalInput")
    out_d = nc.dram_tensor("logits", [T, VOCAB], F32, kind="ExternalOutput")

    wx_r, wy_r, enc_r = wx_d.ap(), wy_d.ap(), enc_d.ap()
    wte_r = wte_d.ap().rearrange("(c p) d -> p c d", p=128)
    ro_r = ro_d.ap().rearrange("(c p) d -> p c d", p=128)

    with tile.TileContext(nc) as tc:
        with tc.tile_pool(name="persist", bufs=1) as pp, \
             tc.tile_pool(name="wq", bufs=2) as wq, \
             tc.tile_pool(name="blk", bufs=8) as blkp, \
             tc.tile_pool(name="sc", bufs=18) as scp, \
             tc.tile_pool(name="st", bufs=48) as stp, \
             tc.tile_pool(name="ps512", bufs=4, space="PSUM") as ps512, \
             tc.tile_pool(name="ps256", bufs=4, space="PSUM") as ps256:

            vT = [pp.tile([128, T], F32R, name=f"vT{c}", tag=f"vT{c}") for c in range(2)]
            vN = pp.tile([128, NTB, D], F32, name="vN", tag="vN")
            vNr = pp.tile([128, NTB, D], F32R, name="vNr", tag="vNr")
            qrT = [pp.tile([128, T], F32R, name=f"qrT{c}", tag=f"qrT{c}") for c in range(2)]
            lnaT = [pp.tile([128, T], F32R, name=f"lnaT{c}", tag=f"lnaT{c}") for c in range(2)]
            updF = pp.tile([128, NTB * D], F32, name="updF", tag="updF")
            _updv = updF.rearrange("p (o d) -> p o d", d=D)

            def updA(tb):
                return _updv[:, tb, :]
            cosT = pp.tile([128, T], F32, name="cosT", tag="cosT")
            sinT = pp.tile([128, T], F32, name="sinT", tag="sinT")
            ropesc = pp.tile([128, TS], F32, name="ropesc", tag="ropesc")
            maskb = pp.tile([128, 1024], BF16, name="maskb", tag="maskb")

            ident = pp.tile([128, 128], F32, name="ident", tag="ident")
            iota_f = pp.tile([128, 2], F32, name="iota_f", tag="iota_f")

            nc.sync.dma_start(ident[:], ident_d.ap())
            nc.sync.dma_start(maskb[:], mask_d.ap())
            nc.sync.dma_start(cosT[:], cos_d.ap())
            nc.sync.dma_start(sinT[:], sin_d.ap())

            copy_flip = [0]

            def copy_any(dst, src):
                # alternate PSUM->SBUF copies between ACT and DVE
                copy_flip[0] ^= 1
                if copy_flip[0]:
                    nc.scalar.copy(dst, src)
                else:
                    nc.vector.tensor_copy(dst, src)

            def mm(psum, lhsT, rhs, start, stop):
                nc.tensor.matmul(psum, lhsT, rhs, start=start, stop=stop)

            def tr128(dst, src):
                pst = ps512.tile([128, 512], F32, name="pst", tag="ps512")
                nc.tensor.transpose(pst[:, :128], src, ident[:])
                copy_any(dst, pst[:, :128])

            def ln_nat_multi(items):
                """Batched LayerNorm over free dim (256): items is a list of
                (src, dst, sums_or_None). Emitted stage-wise across items so
                the per-item chains pipeline through the in-order ACT/DVE
                queues instead of serializing head-of-line."""
                n = len(items)
                sums_l, negmean_l, sqs_l, rstd_l, negmurs_l = [], [], [], [], []
                for src, dst, sums in items:
                    if sums is None:
                        sums = stp.tile([128, 1], F32, name="s1", tag="st")
                        nc.vector.reduce_sum(sums, src, axis=AXX)
                    sums_l.append(sums)
                for i in range(n):
                    negmean = stp.tile([128, 1], F32, name="negmean", tag="st")
                    nc.vector.tensor_scalar_mul(negmean, sums_l[i], -1.0 / D)
                    negmean_l.append(negmean)
                for i in range(n):
                    sq = scp.tile([128, D], F32, name="sq", tag="sc")
                    sqs = stp.tile([128, 1], F32, name="sqs", tag="st")
                    nc.scalar.activation(sq, items[i][0], ACTF.Square,
                                         bias=negmean_l[i], scale=1.0, accum_out=sqs)
                    sqs_l.append(sqs)
                veps_l = []
                for i in range(n):
                    veps = stp.tile([128, 1], F32, name="veps", tag="st")
                    nc.vector.tensor_scalar(veps, sqs_l[i], 1.0 / D, EPS,
                                            op0=ALU.mult, op1=ALU.add)
                    veps_l.append(veps)
                sqv_l = []
                for i in range(n):
                    sqv = stp.tile([128, 1], F32, name="sqv", tag="st")
                    nc.scalar.sqrt(sqv, veps_l[i])
                    sqv_l.append(sqv)
                for i in range(n):
                    rstd = stp.tile([128, 1], F32, name="rstd", tag="st")
                    nc.vector.reciprocal(rstd, sqv_l[i])
                    rstd_l.append(rstd)
                for i in range(n):
                    negmurs = stp.tile([128, 1], F32, name="negmurs", tag="st")
                    nc.vector.tensor_tensor(negmurs, negmean_l[i], rstd_l[i],
                                            op=ALU.mult)
                    negmurs_l.append(negmurs)
                for i in range(n):
                    nc.scalar.activation(items[i][1], items[i][0], ACTF.Identity,
                                         bias=negmurs_l[i], scale=rstd_l[i])

            def ln_nat(src, dst, sums=None):
                ln_nat_multi([(src, dst, sums)])

            def rope_si(si):
                # qrT[:, si block] = rope(vT[:, si block]); runs on the
                # otherwise-idle GPSIMD engine (SBUF-only operands)
                sl = slice(si * TS, (si + 1) * TS)
                nc.gpsimd.tensor_tensor(qrT[0][:, sl], vT[0][:, sl], cosT[:, sl],
                                        op=ALU.mult)
                nc.gpsimd.tensor_tensor(ropesc[:], vT[1][:, sl], sinT[:, sl],
                                        op=ALU.mult)
                nc.gpsimd.tensor_tensor(qrT[0][:, sl], qrT[0][:, sl], ropesc[:],
                                        op=ALU.subtract)
                nc.gpsimd.tensor_tensor(qrT[1][:, sl], vT[1][:, sl], cosT[:, sl],
                                        op=ALU.mult)
                nc.gpsimd.tensor_tensor(ropesc[:], vT[0][:, sl], sinT[:, sl],
                                        op=ALU.mult)
                nc.gpsimd.tensor_tensor(qrT[1][:, sl], qrT[1][:, sl], ropesc[:],
                                        op=ALU.add)

            # readout weights live in a persistent tile so the readout can
            # interleave with the last layer's cphase
            ro_s = pp.tile([128, 2, D], F32, name="ro_s", tag="ro_s")
            nc.sync.dma_start(ro_s[:], ro_r)

            def vmaint_si(si, layer):
                # PE-side per-si maintenance after cphase(si) wrote vN:
                # either rebuild vT (+ rope for the next layer), or run the
                # readout on the last layer (batched across the 4 tbs).
                tbs = list(range(si * 4, si * 4 + 4))
                if layer < layers - 1:
                    for tb in tbs:
                        for c in range(2):
                            tr128(vT[c][:, tb * 128:(tb + 1) * 128],
                                  vN[:, tb, c * 128:(c + 1) * 128])
                    rope_si(si)
                else:
                    vvs = {}
                    for tb in tbs:
                        vv = scp.tile([128, 2, 128], F32, name="vv", tag="sc")
                        for c in range(2):
                            tr128(vv[:, c, :], vN[:, tb, c * 128:(c + 1) * 128])
                        vvs[tb] = vv
                    los = {}
                    for tb in tbs:
                        psR = ps512.tile([128, 512], F32, name="psR", tag="ps512")
                        for c in range(2):
                            mm(psR[:, :D], vvs[tb][:, c, :], ro_s[:, c, :],
                               start=(c == 0), stop=(c == 1))
                        lo = scp.tile([128, VOCAB], F32, name="lo", tag="sc")
                        copy_any(lo[:], psR[:, :D])
                        los[tb] = lo
                    for tb in tbs:
                        nc.sync.dma_start(out_d.ap()[tb * 128:(tb + 1) * 128, :],
                                          los[tb][:])

            # ---------------- embedding: v = ln(wte[idx]) ----------------
            idx_b = lnaT[0]  # scratch alias
            nc.sync.dma_start(idx_b[:], idx_d.ap().partition_broadcast(128))
            wte_s = blkp.tile([128, 2, D], F32, name="wte_s", tag="blk")
            nc.sync.dma_start(wte_s[:], wte_r)
            iota_i = pp.tile([128, 2], I32, name="iota_i", tag="iota_i")
            for c in range(2):
                nc.gpsimd.iota(iota_i[:, c:c + 1], pattern=[[1, 1]], base=c * 128,
                               channel_multiplier=1)
            nc.vector.tensor_copy(iota_f[:], iota_i[:])
            onehot = [updF[:, 0:T], updF[:, T:2 * T]]  # scratch alias (F32)
            for si in range(NSUP):
                sl = slice(si * TS, (si + 1) * TS)
                for c in range(2):
                    nc.vector.tensor_scalar(onehot[c][:, sl], idx_b[:, sl],
                                            iota_f[:, c:c + 1], None,
                                            op0=ALU.is_equal)
                psAs = {}
                for tb in range(si * 4, si * 4 + 4):
                    psA = ps256.tile([128, D], F32, name="psE", tag="ps256")
                    for c in range(2):
                        mm(psA, onehot[c][:, tb * 128:(tb + 1) * 128], wte_s[:, c, :],
                           start=(c == 0), stop=(c == 1))
                    psAs[tb] = psA
                ln_nat_multi([(psAs[tb], vN[:, tb, :], None)
                              for tb in range(si * 4, si * 4 + 4)])
                for tb in range(si * 4, si * 4 + 4):
                    nc.vector.tensor_copy(vNr[:, tb, :], vN[:, tb, :])
                for tb in range(si * 4, si * 4 + 4):
                    for c in range(2):
                        tr128(vT[c][:, tb * 128:(tb + 1) * 128],
                              vN[:, tb, c * 128:(c + 1) * 128])
                rope_si(si)

            # ---------------- layers ----------------
            if not stream_weights:
                wxq0 = wq.tile([128, 2, N // NQ], F32R, name="wxq", tag="wxq")
                nc.sync.dma_start(wxq0[:], wx_r[:, :, 0:N // NQ])
                wyq0 = wq.tile([128, 2, N // NQ], F32R, name="wyq", tag="wyq")
                nc.sync.dma_start(wyq0[:], wy_r[:, :, 0:N // NQ])
                encq0 = wq.tile([128, NCHQ, D], F32R, name="encq", tag="encq")
                nc.sync.dma_start(encq0[:], enc_r[:, 0:NCHQ, :])
            pend_vmaint_h = [None]
            for layer in range(layers):
                # (qrT for this layer was produced by the previous layer's
                #  interleaved rope_si calls, or by the embedding epilogue)
                pend_vmaint = pend_vmaint_h[0]
                pend_vmaint_h[0] = None

                # --- attention + LN(a) -> lnaT ---
                # psE runs one sc ahead of psA (skew-1) so PE covers the
                # DVE masking latency; LN(a) transposes land one si late so
                # their LN-chain deps are satisfied; the previous layer's
                # si=3 vT/rope maintenance is emitted after si=0's matmuls.
                lna_pend = {}

                def emit_energy(si, sc, eTs):
                    psE = ps512.tile([128, TS], F32, name="psE", tag="ps512")
                    for c in range(2):
                        mm(psE, qrT[c][:, sc * 128:(sc + 1) * 128],
                           qrT[c][:, si * TS:(si + 1) * TS],
                           start=(c == 0), stop=(c == 1))
                    eT = blkp.tile([128, TS], F32R, name="eT", tag="blk")
                    k = sc - 4 * si
                    if k < 0:
                        copy_any(eT[:], psE[:])
                    else:
                        nc.vector.tensor_tensor(
                            eT[:], psE[:], maskb[:, 384 - k * 128: 896 - k * 128],
                            op=ALU.mult)
                    eTs[sc] = eT

                def emit_a(si, sc, eTs, psA):
                    eT = eTs.pop(sc)
                    for tb4 in range(4):
                        tb = si * 4 + tb4
                        if sc <= tb:
                            mm(psA[tb4], eT[:, tb4 * 128:(tb4 + 1) * 128],
                               vNr[:, sc, :], start=(sc == 0), stop=(sc == tb))

                def emit_lna_tr(si):
                    for tb4 in range(4):
                        tb = si * 4 + tb4
                        lna_n = lna_pend.pop((si, tb4))
                        for c in range(2):
                            tr128(lnaT[c][:, tb * 128:(tb + 1) * 128],
                                  lna_n[:, c * 128:(c + 1) * 128])

                for si in range(NSUP if attn else 0):
                    psA = [ps256.tile([128, D], F32, name="psA", tag="ps256")
                           for _ in range(4)]
                    nsc = 4 * si + 4
                    eTs = {}
                    emit_energy(si, 0, eTs)
                    for sc in range(1, nsc):
                        emit_energy(si, sc, eTs)
                        emit_a(si, sc - 1, eTs, psA)
                    emit_a(si, nsc - 1, eTs, psA)
                    lna_items = []
                    for tb4 in range(4):
                        lna_n = scp.tile([128, D], F32, name="lna_n", tag="sc")
                        lna_items.append((psA[tb4], lna_n, None))
                        lna_pend[(si, tb4)] = lna_n
                    ln_nat_multi(lna_items)
                    if si == 1 and pend_vmaint is not None:
                        pend_vmaint()
                    if si >= 1:
                        emit_lna_tr(si - 1)
                if attn:
                    emit_lna_tr(NSUP - 1)

                # --- MLP over N eighths ---
                upd_sums = {}
                for q in range(NQ):
                    if stream_weights:
                        qs = slice(q * (N // NQ), (q + 1) * (N // NQ))
                        wxq = wq.tile([128, 2, N // NQ], F32R, name="wxq", tag="wxq")
                        nc.sync.dma_start(wxq[:], wx_r[:, :, qs])
                        wyq = wq.tile([128, 2, N // NQ], F32R, name="wyq", tag="wyq")
                        nc.sync.dma_start(wyq[:], wy_r[:, :, qs])
                        encq = wq.tile([128, NCHQ, D], F32R, name="encq", tag="encq")
                        nc.sync.dma_start(encq[:], enc_r[:, q * NCHQ:(q + 1) * NCHQ, :])
                    else:
                        wxq, wyq, encq = wxq0, wyq0, encq0
                    for si in range(NSUP):
                        sl = slice(si * TS, (si + 1) * TS)
                        ln_src = lnaT if attn else qrT
                        psU = [ps256.tile([128, D], F32, name="psU", tag="ps256")
                               for _ in range(4)]
                        ys = {}

                        def emit_xy(nch):
                            psX = ps512.tile([128, TS], F32, name="psX", tag="ps512")
                            psY = ps512.tile([128, TS], F32, name="psY", tag="ps512")
                            ns = slice(nch * 128, (nch + 1) * 128)
                            for c in range(2):
                                mm(psX, wxq[:, c, ns], vT[c][:, sl],
                                   start=(c == 0), stop=(c == 1))
                                mm(psY, wyq[:, c, ns], ln_src[c][:, sl],
                                   start=(c == 0), stop=(c == 1))
                            xr = blkp.tile([128, TS], F32, name="xr", tag="blk")
                            nc.scalar.activation(xr, psX, ACTF.Relu)
                            ysb = blkp.tile([128, TS], F32R, name="ysb", tag="blk")
                            nc.vector.scalar_tensor_tensor(
                                ysb, psY, 0.0, xr, op0=ALU.max, op1=ALU.mult)
                            ys[nch] = ysb

                        def emit_u(nch):
                            for tb4 in range(4):
                                t4 = slice(tb4 * 128, (tb4 + 1) * 128)
                                mm(psU[tb4], ys[nch][:, t4], encq[:, nch, :],
                                   start=(nch == 0), stop=(nch == NCHQ - 1))
                            del ys[nch]

                        # skew-1 software pipeline: psU(nch-1) lands after
                        # psX/psY(nch) so the in-order PE stream never waits
                        # on the ACT relu -> DVE mult chain
                        for nch in range(NCHQ):
                            emit_xy(nch)
                            if nch >= 1:
                                emit_u(nch - 1)
                        emit_u(NCHQ - 1)
                        for tb4 in range(4):
                            tb = si * 4 + tb4
                            dst = updA(tb)
                            if q == 0:
                                nc.scalar.copy(dst, psU[tb4])
                            elif q < NQ - 1:
                                nc.vector.tensor_tensor(dst, psU[tb4], dst, op=ALU.add)
                            else:
                                s2 = stp.tile([128, 1], F32, name="s2", tag="st")
                                nc.vector.scalar_tensor_tensor(
                                    dst, psU[tb4], 0.0, dst, op0=ALU.add,
                                    op1=ALU.add, accum_out=s2)
                                upd_sums[tb] = s2
                        if q == NQ - 1 and cphase:
                            # --- interleaved cphase for this si (DVE/ACT
                            # only), stage-batched across the 4 tbs:
                            # v = ln(v + ln(update)) -> vN, vNr ---
                            tbs = list(range(si * 4, si * 4 + 4))
                            lnus = {}
                            for tb in tbs:
                                lnus[tb] = scp.tile([128, D], F32, name="lnu",
                                                    tag="sc")
                            ln_nat_multi([(updA(tb), lnus[tb], upd_sums[tb])
                                          for tb in tbs])
                            vmids, s3s = {}, {}
                            for tb in tbs:
                                vmid = scp.tile([128, D], F32, name="vmid", tag="sc")
                                s3 = stp.tile([128, 1], F32, name="s3", tag="st")
                                nc.vector.scalar_tensor_tensor(
                                    vmid, lnus[tb], 0.0, vN[:, tb, :], op0=ALU.add,
                                    op1=ALU.add, accum_out=s3)
                                vmids[tb], s3s[tb] = vmid, s3
                            ln_nat_multi([(vmids[tb], vN[:, tb, :], s3s[tb])
                                          for tb in tbs])
                            if layer < layers - 1:
                                for tb in tbs:
                                    nc.gpsimd.tensor_copy(vNr[:, tb, :],
                                                          vN[:, tb, :])
                            # --- PE-side maintenance for si-1, one si late so
                            # its LN-chain dependencies are already satisfied
                            # when the in-order PE stream reaches it ---
                            if si >= 1:
                                vmaint_si(si - 1, layer)
                        if q == NQ - 1 and cphase and si == NSUP - 1:
                            if layer < layers - 1:
                                # defer into the next layer's attention phase
                                # so it never head-of-line-blocks PE here
                                pend_vmaint_h[0] = (
                                    lambda si=si, layer=layer:
                                    vmaint_si(si, layer))
                            else:
                                vmaint_si(si, layer)  # final readout tail

    nc.compile()
    return nc


_NC_CACHE = {}


def get_nc():
    if "nc" not in _NC_CACHE:
        _NC_CACHE["nc"] = build_nc()
    return _NC_CACHE["nc"]


def make_host_inputs(idx, wte, encoder, decoder_x, decoder_y, readout):
    idx = np.asarray(idx)
    wte = np.asarray(wte, dtype=np.float32)
    encoder = np.asarray(encoder, dtype=np.float32)
    decoder_x = np.asarray(decoder_x, dtype=np.float32)
    decoder_y = np.asarray(decoder_y, dtype=np.float32)
    readout = np.asarray(readout, dtype=np.float32)

    wx = decoder_x.transpose(1, 0, 2).reshape(D, N)
    wy = decoder_y.transpose(1, 0, 2).reshape(D, N)
    # partition-contiguous layouts for fast DMA: [p, c, n] with d = c*128 + p
    wx = np.ascontiguousarray(wx.reshape(2, 128, N).transpose(1, 0, 2))
    wy = np.ascontiguousarray(wy.reshape(2, 128, N).transpose(1, 0, 2))
    # enc: [p, o, d] with n = o*128 + p
    enc_s = np.ascontiguousarray(encoder.reshape(N // 128, 128, D).transpose(1, 0, 2))

    inv_freq = 1.0 / (10000.0 ** (np.arange(0, D, 2, dtype=np.float32) / D))  # [128]
    t = np.arange(T, dtype=np.float32)
    freqsT = inv_freq[:, None] * t[None, :]                   # [128, T]
    cosT = np.cos(freqsT).astype(np.float32)
    sinT = np.sin(freqsT).astype(np.float32)

    s_idx = np.arange(128, dtype=np.int32)[:, None]
    c_idx = np.arange(1024, dtype=np.int32)[None, :]
    maskbig = (s_idx <= c_idx - 384).astype(ml_dtypes.bfloat16)

    in_maps = []
    for b in range(B):
        in_maps.append({
            "idxf": idx[b].astype(np.float32).reshape(1, T),
            "wte": wte,
            "wx": wx,
            "wy": wy,
            "enc": enc_s,
            "ro": readout,
            "cosT": cosT,
            "sinT": sinT,
            "maskbig": maskbig,
            "identm": np.eye(128, dtype=np.float32),
        })
    return in_maps


def kernel(idx, wte, encoder, decoder_x, decoder_y, readout):
    nc = get_nc()
    in_maps = make_host_inputs(idx, wte, encoder, decoder_x, decoder_y, readout)
    res = bass_utils.run_bass_kernel_spmd(nc, in_maps, core_ids=list(range(B)))
    out = np.stack([res.results[b]["logits"] for b in range(B)], axis=0)
    return out.astype(np.float32)


# revision 57
# speedup vs baseline: 1.0405x; 1.0405x over previous
"""Trainium2 Bass kernel for the BDH dense-transformer problem.

Sharding: data-parallel over B=8 across the 8 NeuronCores (one batch
element per core, no collectives). Each core runs the full 6-layer
network on its [T=2048, D=256] slice.

Per-core program. Matmul precision strategy:
  - the per-layer FLOP bulk (attention energy/a, MLP x/y/update) runs
    in float32r: 1 cyc/row on the PE when the output free dim is
    >= 256 (true for all matmuls here), vs 4 cyc/row for fp32 and
    3 cyc/row for the bf16x2 3-pass split scheme. No host splits and
    no DVE split work. All tensors feeding an f32r matmul are declared
    float32r so writes round appropriately (BIR verifier requirement).
  - precision recovery: the residual stream vN and the update
    accumulator updW stay full fp32 (vNr is a rounded F32R mirror used
    only as the attention a-matmul input), and the run-once embedding
    and readout matmuls are full fp32. Only per-layer matmul-input
    roundings remain.
Structure:
  - token embedding via one-hot matmul (iota + is_equal + PE)
  - v kept in both layouts: vT [D,T] (f32r) and vN [T,D] (fp32)
  - causal linear attention block-wise: energyT = qr@qr^T per
    [s128, t512] block (PSUM), bf16-mask multiply, then aN accumulated
    in PSUM over s-chunks
  - LayerNorms in natural layout with fused ACT Square/Identity
    (per-partition scale+bias + accum_out row sums)
  - MLP streamed over N in eighths (fp32 weights DMA'd per layer in
    host-pre-shuffled partition-contiguous layouts),
    relu(x)*relu(y) fused via scalar_tensor_tensor, update accumulated
    in PSUM then SBUF
  - PE 128x128 transposes maintain both v layouts
"""

import math

import numpy as np
import ml_dtypes

import concourse.bass as bass
import concourse.tile as tile
from concourse import bacc, mybir
from concourse import bass_utils

F32 = mybir.dt.float32
F32R = mybir.dt.float32r
BF16 = mybir.dt.bfloat16
I32 = mybir.dt.int32
ALU = mybir.AluOpType
ACTF = mybir.ActivationFunctionType
AXX = mybir.AxisListType.X

B, T, D, N, H, VOCAB, L = 8, 2048, 256, 8192, 4, 256, 6
EPS = 1e-5
TS = 512          # t-super width
NSUP = T // TS    # 4
NTB = T // 128    # 16
NQ = 8            # weight chunks along N
NCHQ = N // 128 // NQ  # 8 n-chunks per weight chunk


def build_nc(layers=L, stream_weights=True, attn=True, cphase=True):
    nc = bacc.Bacc("TRN2", target_bir_lowering=False, debug=False)

    idx_d = nc.dram_tensor("idxf", [1, T], F32R, kind="ExternalInput")
    wte_d = nc.dram_tensor("wte", [VOCAB, D], F32, kind="ExternalInput")
    wx_d = nc.dram_tensor("wx", [128, 2, N], F32R, kind="ExternalInput")
    wy_d = nc.dram_tensor("wy", [128, 2, N], F32R, kind="ExternalInput")
    enc_d = nc.dram_tensor("enc", [128, N // 128, D], F32R, kind="ExternalInput")
    ro_d = nc.dram_tensor("ro", [D, VOCAB], F32, kind="ExternalInput")
    cos_d = nc.dram_tensor("cosT", [128, T], F32, kind="ExternalInput")
    sin_d = nc.dram_tensor("sinT", [128, T], F32, kind="ExternalInput")
    mask_d = nc.dram_tensor("maskbig", [128, 1024], BF16, kind="ExternalInput")
    ident_d = nc.dram_tensor("identm", [128, 128], F32, kind="ExternalInput")
    out_d = nc.dram_tensor("logits", [T, VOCAB], F32, kind="ExternalOutput")

    wx_r, wy_r, enc_r = wx_d.ap(), wy_d.ap(), enc_d.ap()
    wte_r = wte_d.ap().rearrange("(c p) d -> p c d", p=128)
    ro_r = ro_d.ap().rearrange("(c p) d -> p c d", p=128)

    with tile.TileContext(nc) as tc:
        with tc.tile_pool(name="persist", bufs=1) as pp, \
             tc.tile_pool(name="wq", bufs=2) as wq, \
             tc.tile_pool(name="blk", bufs=8) as blkp, \
             tc.tile_pool(name="sc", bufs=18) as scp, \
             tc.tile_pool(name="st", bufs=48) as stp, \
             tc.tile_pool(name="ps512", bufs=4, space="PSUM") as ps512, \
             tc.tile_pool(name="ps256", bufs=4, space="PSUM") as ps256:

            vT = [pp.tile([128, T], F32R, name=f"vT{c}", tag=f"vT{c}") for c in range(2)]
            vN = pp.tile([128, NTB, D], F32, name="vN", tag="vN")
            vNr = pp.tile([128, NTB, D], F32R, name="vNr", tag="vNr")
            qrT = [pp.tile([128, T], F32R, name=f"qrT{c}", tag=f"qrT{c}") for c in range(2)]
            lnaT = [pp.tile([128, T], F32R, name=f"lnaT{c}", tag=f"lnaT{c}") for c in range(2)]
            updF = pp.tile([128, NTB * D], F32, name="updF", tag="updF")
            _updv = updF.rearrange("p (o d) -> p o d", d=D)

            def updA(tb):
                return _updv[:, tb, :]
            cosT = pp.tile([128, T], F32, name="cosT", tag="cosT")
            sinT = pp.tile([128, T], F32, name="sinT", tag="sinT")
            ropesc = pp.tile([128, TS], F32, name="ropesc", tag="ropesc")
            maskb = pp.tile([128, 1024], BF16, name="maskb", tag="maskb")

            ident = pp.tile([128, 128], F32, name="ident", tag="ident")
            iota_f = pp.tile([128, 2], F32, name="iota_f", tag="iota_f")

            nc.sync.dma_start(ident[:], ident_d.ap())
            nc.sync.dma_start(maskb[:], mask_d.ap())
            nc.sync.dma_start(cosT[:], cos_d.ap())
            nc.sync.dma_start(sinT[:], sin_d.ap())

            copy_flip = [0]

            def copy_any(dst, src):
                # alternate PSUM->SBUF copies between ACT and DVE
                copy_flip[0] ^= 1
                if copy_flip[0]:
                    nc.scalar.copy(dst, src)
                else:
                    nc.vector.tensor_copy(dst, src)

            def mm(psum, lhsT, rhs, start, stop):
                nc.tensor.matmul(psum, lhsT, rhs, start=start, stop=stop)

            def tr128(dst, src):
                pst = ps512.tile([128, 512], F32, name="pst", tag="ps512")
                nc.tensor.transpose(pst[:, :128], src, ident[:])
                copy_any(dst, pst[:, :128])

            def ln_nat_multi(items):
                """Batched LayerNorm over free dim (256): items is a list of
                (src, dst, sums_or_None). Emitted stage-wise across items so
                the per-item chains pipeline through the in-order ACT/DVE
                queues instead of serializing head-of-line."""
                n = len(items)
                sums_l, negmean_l, sqs_l, rstd_l, negmurs_l = [], [], [], [], []
                for src, dst, sums in items:
                    if sums is None:
                        sums = stp.tile([128, 1], F32, name="s1", tag="st")
                        nc.vector.reduce_sum(sums, src, axis=AXX)
                    sums_l.append(sums)
                for i in range(n):
                    negmean = stp.tile([128, 1], F32, name="negmean", tag="st")
                    nc.vector.tensor_scalar_mul(negmean, sums_l[i], -1.0 / D)
                    negmean_l.append(negmean)
                for i in range(n):
                    sq = scp.tile([128, D], F32, name="sq", tag="sc")
                    sqs = stp.tile([128, 1], F32, name="sqs", tag="st")
                    nc.scalar.activation(sq, items[i][0], ACTF.Square,
                                         bias=negmean_l[i], scale=1.0, accum_out=sqs)
                    sqs_l.append(sqs)
                veps_l = []
                for i in range(n):
                    veps = stp.tile([128, 1], F32, name="veps", tag="st")
                    nc.vector.tensor_scalar(veps, sqs_l[i], 1.0 / D, EPS,
                                            op0=ALU.mult, op1=ALU.add)
                    veps_l.append(veps)
                sqv_l = []
                for i in range(n):
                    sqv = stp.tile([128, 1], F32, name="sqv", tag="st")
                    nc.scalar.sqrt(sqv, veps_l[i])
                    sqv_l.append(sqv)
                for i in range(n):
                    rstd = stp.tile([128, 1], F32, name="rstd", tag="st")
                    nc.vector.reciprocal(rstd, sqv_l[i])
                    rstd_l.append(rstd)
                for i in range(n):
                    negmurs = stp.tile([128, 1], F32, name="negmurs", tag="st")
                    nc.vector.tensor_tensor(negmurs, negmean_l[i], rstd_l[i],
                                            op=ALU.mult)
                    negmurs_l.append(negmurs)
                for i in range(n):
                    nc.scalar.activation(items[i][1], items[i][0], ACTF.Identity,
                                         bias=negmurs_l[i], scale=rstd_l[i])

            def ln_nat(src, dst, sums=None):
                ln_nat_multi([(src, dst, sums)])

            def rope_si(si):
                # qrT[:, si block] = rope(vT[:, si block]); runs on the
                # otherwise-idle GPSIMD engine (SBUF-only operands)
                sl = slice(si * TS, (si + 1) * TS)
                nc.gpsimd.tensor_tensor(qrT[0][:, sl], vT[0][:, sl], cosT[:, sl],
                                        op=ALU.mult)
                nc.gpsimd.tensor_tensor(ropesc[:], vT[1][:, sl], sinT[:, sl],
                                        op=ALU.mult)
                nc.gpsimd.tensor_tensor(qrT[0][:, sl], qrT[0][:, sl], ropesc[:],
                                        op=ALU.subtract)
                nc.gpsimd.tensor_tensor(qrT[1][:, sl], vT[1][:, sl], cosT[:, sl],
                                        op=ALU.mult)
                nc.gpsimd.tensor_tensor(ropesc[:], vT[0][:, sl], sinT[:, sl],
                                        op=ALU.mult)
                nc.gpsimd.tensor_tensor(qrT[1][:, sl], qrT[1][:, sl], ropesc[:],
                                        op=ALU.add)

            # readout weights live in a persistent tile so the readout can
            # interleave with the last layer's cphase
            ro_s = pp.tile([128, 2, D], F32, name="ro_s", tag="ro_s")
            nc.sync.dma_start(ro_s[:], ro_r)

            def vmaint_si(si, layer):
                # PE-side per-si maintenance after cphase(si) wrote vN:
                # either rebuild vT (+ rope for the next layer), or run the
                # readout on the last layer (batched across the 4 tbs).
                tbs = list(range(si * 4, si * 4 + 4))
                if layer < layers - 1:
                    for tb in tbs:
                        for c in range(2):
                            tr128(vT[c][:, tb * 128:(tb + 1) * 128],
                                  vN[:, tb, c * 128:(c + 1) * 128])
                    rope_si(si)
                else:
                    vvs = {}
                    for tb in tbs:
                        vv = scp.tile([128, 2, 128], F32, name="vv", tag="sc")
                        for c in range(2):
                            tr128(vv[:, c, :], vN[:, tb, c * 128:(c + 1) * 128])
                        vvs[tb] = vv
                    los = {}
                    for tb in tbs:
                        psR = ps512.tile([128, 512], F32, name="psR", tag="ps512")
                        for c in range(2):
                            mm(psR[:, :D], vvs[tb][:, c, :], ro_s[:, c, :],
                               start=(c == 0), stop=(c == 1))
                        lo = scp.tile([128, VOCAB], F32, name="lo", tag="sc")
                        copy_any(lo[:], psR[:, :D])
                        los[tb] = lo
                    for tb in tbs:
                        nc.sync.dma_start(out_d.ap()[tb * 128:(tb + 1) * 128, :],
                                          los[tb][:])

            # ---------------- embedding: v = ln(wte[idx]) ----------------
            idx_b = lnaT[0]  # scratch alias
            nc.sync.dma_start(idx_b[:], idx_d.ap().partition_broadcast(128))
            wte_s = blkp.tile([128, 2, D], F32, name="wte_s", tag="blk")
            nc.sync.dma_start(wte_s[:], wte_r)
            iota_i = pp.tile([128, 2], I32, name="iota_i", tag="iota_i")
            for c in range(2):
                nc.gpsimd.iota(iota_i[:, c:c + 1], pattern=[[1, 1]], base=c * 128,
                               channel_multiplier=1)
            nc.vector.tensor_copy(iota_f[:], iota_i[:])
            onehot = [updF[:, 0:T], updF[:, T:2 * T]]  # scratch alias (F32)
            for si in range(NSUP):
                sl = slice(si * TS, (si + 1) * TS)
                for c in range(2):
                    nc.vector.tensor_scalar(onehot[c][:, sl], idx_b[:, sl],
                                            iota_f[:, c:c + 1], None,
                                            op0=ALU.is_equal)
                psAs = {}
                for tb in range(si * 4, si * 4 + 4):
                    psA = ps256.tile([128, D], F32, name="psE", tag="ps256")
                    for c in range(2):
                        mm(psA, onehot[c][:, tb * 128:(tb + 1) * 128], wte_s[:, c, :],
                           start=(c == 0), stop=(c == 1))
                    psAs[tb] = psA
                ln_nat_multi([(psAs[tb], vN[:, tb, :], None)
                              for tb in range(si * 4, si * 4 + 4)])
                for tb in range(si * 4, si * 4 + 4):
                    nc.vector.tensor_copy(vNr[:, tb, :], vN[:, tb, :])
                for tb in range(si * 4, si * 4 + 4):
                    for c in range(2):
                        tr128(vT[c][:, tb * 128:(tb + 1) * 128],
                              vN[:, tb, c * 128:(c + 1) * 128])
                rope_si(si)

            # ---------------- layers ----------------
            if not stream_weights:
                wxq0 = wq.tile([128, 2, N // NQ], F32R, name="wxq", tag="wxq")
                nc.sync.dma_start(wxq0[:], wx_r[:, :, 0:N // NQ])
                wyq0 = wq.tile([128, 2, N // NQ], F32R, name="wyq", tag="wyq")
                nc.sync.dma_start(wyq0[:], wy_r[:, :, 0:N // NQ])
                encq0 = wq.tile([128, NCHQ, D], F32R, name="encq", tag="encq")
                nc.sync.dma_start(encq0[:], enc_r[:, 0:NCHQ, :])
            pend_vmaint_h = [None]
            for layer in range(layers):
                # (qrT for this layer was produced by the previous layer's
                #  interleaved rope_si calls, or by the embedding epilogue)
                pend_vmaint = pend_vmaint_h[0]
                pend_vmaint_h[0] = None

                # --- attention + LN(a) -> lnaT ---
                # psE runs one sc ahead of psA (skew-1) so PE covers the
                # DVE masking latency; LN(a) transposes land one si late so
                # their LN-chain deps are satisfied; the previous layer's
                # si=3 vT/rope maintenance is emitted after si=0's matmuls.
                lna_pend = {}

                def emit_energy(si, sc, eTs):
                    psE = ps512.tile([128, TS], F32, name="psE", tag="ps512")
                    for c in range(2):
                        mm(psE, qrT[c][:, sc * 128:(sc + 1) * 128],
                           qrT[c][:, si * TS:(si + 1) * TS],
                           start=(c == 0), stop=(c == 1))
                    eT = blkp.tile([128, TS], F32R, name="eT", tag="blk")
                    k = sc - 4 * si
                    if k < 0:
                        copy_any(eT[:], psE[:])
                    else:
                        nc.vector.tensor_tensor(
                            eT[:], psE[:], maskb[:, 384 - k * 128: 896 - k * 128],
                            op=ALU.mult)
                    eTs[sc] = eT

                def emit_a(si, sc, eTs, psA):
                    eT = eTs.pop(sc)
                    for tb4 in range(4):
                        tb = si * 4 + tb4
                        if sc <= tb:
                            mm(psA[tb4], eT[:, tb4 * 128:(tb4 + 1) * 128],
                               vNr[:, sc, :], start=(sc == 0), stop=(sc == tb))

                def emit_lna_tr(si):
                    for tb4 in range(4):
                        tb = si * 4 + tb4
                        lna_n = lna_pend.pop((si, tb4))
                        for c in range(2):
                            tr128(lnaT[c][:, tb * 128:(tb + 1) * 128],
                                  lna_n[:, c * 128:(c + 1) * 128])

                for si in range(NSUP if attn else 0):
                    psA = [ps256.tile([128, D], F32, name="psA", tag="ps256")
                           for _ in range(4)]
                    nsc = 4 * si + 4
                    eTs = {}
                    emit_energy(si, 0, eTs)
                    emit_energy(si, 1, eTs)
                    for sc in range(2, nsc):
                        emit_energy(si, sc, eTs)
                        emit_a(si, sc - 2, eTs, psA)
                    emit_a(si, nsc - 2, eTs, psA)
                    emit_a(si, nsc - 1, eTs, psA)
                    lna_items = []
                    for tb4 in range(4):
                        lna_n = scp.tile([128, D], F32, name="lna_n", tag="sc")
                        lna_items.append((psA[tb4], lna_n, None))
                        lna_pend[(si, tb4)] = lna_n
                    ln_nat_multi(lna_items)
                    if si == 1 and pend_vmaint is not None:
                        pend_vmaint()
                    if si >= 1:
                        emit_lna_tr(si - 1)
                if attn:
                    emit_lna_tr(NSUP - 1)

                # --- MLP over N eighths: one flat skew-1 pipeline over
                # (q, si, nch) so the in-order PE stream never drains at
                # (q, si) boundaries ---
                upd_sums = {}
                ln_src = lnaT if attn else qrT
                w_cache = {}

                def fetch_w(q):
                    if q in w_cache:
                        return w_cache[q]
                    if stream_weights:
                        qs = slice(q * (N // NQ), (q + 1) * (N // NQ))
                        wxq = wq.tile([128, 2, N // NQ], F32R, name="wxq", tag="wxq")
                        nc.sync.dma_start(wxq[:], wx_r[:, :, qs])
                        wyq = wq.tile([128, 2, N // NQ], F32R, name="wyq", tag="wyq")
                        nc.sync.dma_start(wyq[:], wy_r[:, :, qs])
                        encq = wq.tile([128, NCHQ, D], F32R, name="encq", tag="encq")
                        nc.sync.dma_start(encq[:], enc_r[:, q * NCHQ:(q + 1) * NCHQ, :])
                        w_cache[q] = (wxq, wyq, encq)
                    else:
                        w_cache[q] = (wxq0, wyq0, encq0)
                    return w_cache[q]

                psU_cache = {}

                def get_psU(q, si):
                    if (q, si) not in psU_cache:
                        psU_cache[(q, si)] = [
                            ps256.tile([128, D], F32, name="psU", tag="ps256")
                            for _ in range(4)]
                    return psU_cache[(q, si)]

                ys = {}

                def emit_xy(q, si, nch):
                    wxq, wyq, encq = fetch_w(q)
                    sl = slice(si * TS, (si + 1) * TS)
                    psX = ps512.tile([128, TS], F32, name="psX", tag="ps512")
                    psY = ps512.tile([128, TS], F32, name="psY", tag="ps512")
                    ns = slice(nch * 128, (nch + 1) * 128)
                    for c in range(2):
                        mm(psX, wxq[:, c, ns], vT[c][:, sl],
                           start=(c == 0), stop=(c == 1))
                        mm(psY, wyq[:, c, ns], ln_src[c][:, sl],
                           start=(c == 0), stop=(c == 1))
                    xr = blkp.tile([128, TS], F32, name="xr", tag="blk")
                    nc.scalar.activation(xr, psX, ACTF.Relu)
                    ysb = blkp.tile([128, TS], F32R, name="ysb", tag="blk")
                    nc.vector.scalar_tensor_tensor(
                        ysb, psY, 0.0, xr, op0=ALU.max, op1=ALU.mult)
                    ys[(q, si, nch)] = ysb

                def emit_adds(q, si):
                    psU = psU_cache.pop((q, si))
                    for tb4 in range(4):
                        tb = si * 4 + tb4
                        dst = updA(tb)
                        if q == 0:
                            nc.scalar.copy(dst, psU[tb4])
                        elif q < NQ - 1:
                            nc.vector.tensor_tensor(dst, psU[tb4], dst, op=ALU.add)
                        else:
                            s2 = stp.tile([128, 1], F32, name="s2", tag="st")
                            nc.vector.scalar_tensor_tensor(
                                dst, psU[tb4], 0.0, dst, op0=ALU.add,
                                op1=ALU.add, accum_out=s2)
                            upd_sums[tb] = s2

                def emit_cphase(si):
                    # interleaved cphase for this si (DVE/ACT only),
                    # stage-batched across the 4 tbs:
                    # v = ln(v + ln(update)) -> vN, vNr
                    tbs = list(range(si * 4, si * 4 + 4))
                    lnus = {}
                    for tb in tbs:
                        lnus[tb] = scp.tile([128, D], F32, name="lnu", tag="sc")
                    ln_nat_multi([(updA(tb), lnus[tb], upd_sums[tb])
                                  for tb in tbs])
                    vmids, s3s = {}, {}
                    for tb in tbs:
                        vmid = scp.tile([128, D], F32, name="vmid", tag="sc")
                        s3 = stp.tile([128, 1], F32, name="s3", tag="st")
                        nc.vector.scalar_tensor_tensor(
                            vmid, lnus[tb], 0.0, vN[:, tb, :], op0=ALU.add,
                            op1=ALU.add, accum_out=s3)
                        vmids[tb], s3s[tb] = vmid, s3
                    ln_nat_multi([(vmids[tb], vN[:, tb, :], s3s[tb])
                                  for tb in tbs])
                    if layer < layers - 1:
                        for tb in tbs:
                            nc.gpsimd.tensor_copy(vNr[:, tb, :], vN[:, tb, :])

                def emit_u(q, si, nch):
                    _, _, encq = fetch_w(q)
                    psU = get_psU(q, si)
                    ysb = ys.pop((q, si, nch))
                    for tb4 in range(4):
                        t4 = slice(tb4 * 128, (tb4 + 1) * 128)
                        mm(psU[tb4], ysb[:, t4], encq[:, nch, :],
                           start=(nch == 0), stop=(nch == NCHQ - 1))
                    if nch == NCHQ - 1:
                        emit_adds(q, si)
                        if q == NQ - 1 and cphase:
                            emit_cphase(si)
                            if si >= 1:
                                vmaint_si(si - 1, layer)
                            if si == NSUP - 1:
                                if layer < layers - 1:
                                    pend_vmaint_h[0] = (
                                        lambda si=si, layer=layer:
                                        vmaint_si(si, layer))
                                else:
                                    vmaint_si(si, layer)  # final readout tail

                prev = None
                for q in range(NQ):
                    for si in range(NSUP):
                        for nch in range(NCHQ):
                            emit_xy(q, si, nch)
                            if prev is not None:
                                emit_u(*prev)
                            prev = (q, si, nch)
                emit_u(*prev)

    nc.compile()
    return nc


_NC_CACHE = {}


def get_nc():
    if "nc" not in _NC_CACHE:
        _NC_CACHE["nc"] = build_nc()
    return _NC_CACHE["nc"]


def make_host_inputs(idx, wte, encoder, decoder_x, decoder_y, readout):
    idx = np.asarray(idx)
    wte = np.asarray(wte, dtype=np.float32)
    encoder = np.asarray(encoder, dtype=np.float32)
    decoder_x = np.asarray(decoder_x, dtype=np.float32)
    decoder_y = np.asarray(decoder_y, dtype=np.float32)
    readout = np.asarray(readout, dtype=np.float32)

    wx = decoder_x.transpose(1, 0, 2).reshape(D, N)
    wy = decoder_y.transpose(1, 0, 2).reshape(D, N)
    # partition-contiguous layouts for fast DMA: [p, c, n] with d = c*128 + p
    wx = np.ascontiguousarray(wx.reshape(2, 128, N).transpose(1, 0, 2))
    wy = np.ascontiguousarray(wy.reshape(2, 128, N).transpose(1, 0, 2))
    # enc: [p, o, d] with n = o*128 + p
    enc_s = np.ascontiguousarray(encoder.reshape(N // 128, 128, D).transpose(1, 0, 2))

    inv_freq = 1.0 / (10000.0 ** (np.arange(0, D, 2, dtype=np.float32) / D))  # [128]
    t = np.arange(T, dtype=np.float32)
    freqsT = inv_freq[:, None] * t[None, :]                   # [128, T]
    cosT = np.cos(freqsT).astype(np.float32)
    sinT = np.sin(freqsT).astype(np.float32)

    s_idx = np.arange(128, dtype=np.int32)[:, None]
    c_idx = np.arange(1024, dtype=np.int32)[None, :]
    maskbig = (s_idx <= c_idx - 384).astype(ml_dtypes.bfloat16)

    in_maps = []
    for b in range(B):
        in_maps.append({
            "idxf": idx[b].astype(np.float32).reshape(1, T),
            "wte": wte,
            "wx": wx,
            "wy": wy,
            "enc": enc_s,
            "ro": readout,
            "cosT": cosT,
            "sinT": sinT,
            "maskbig": maskbig,
            "identm": np.eye(128, dtype=np.float32),
        })
    return in_maps


def kernel(idx, wte, encoder, decoder_x, decoder_y, readout):
    nc = get_nc()
    in_maps = make_host_inputs(idx, wte, encoder, decoder_x, decoder_y, readout)
    res = bass_utils.run_bass_kernel_spmd(nc, in_maps, core_ids=list(range(B)))
    out = np.stack([res.results[b]["logits"] for b in range(B)], axis=0)
    return out.astype(np.float32)


# revision 58
# speedup vs baseline: 1.0434x; 1.0028x over previous
"""Trainium2 Bass kernel for the BDH dense-transformer problem.

Sharding: data-parallel over B=8 across the 8 NeuronCores (one batch
element per core, no collectives). Each core runs the full 6-layer
network on its [T=2048, D=256] slice.

Per-core program. Matmul precision strategy:
  - the per-layer FLOP bulk (attention energy/a, MLP x/y/update) runs
    in float32r: 1 cyc/row on the PE when the output free dim is
    >= 256 (true for all matmuls here), vs 4 cyc/row for fp32 and
    3 cyc/row for the bf16x2 3-pass split scheme. No host splits and
    no DVE split work. All tensors feeding an f32r matmul are declared
    float32r so writes round appropriately (BIR verifier requirement).
  - precision recovery: the residual stream vN and the update
    accumulator updW stay full fp32 (vNr is a rounded F32R mirror used
    only as the attention a-matmul input), and the run-once embedding
    and readout matmuls are full fp32. Only per-layer matmul-input
    roundings remain.
Structure:
  - token embedding via one-hot matmul (iota + is_equal + PE)
  - v kept in both layouts: vT [D,T] (f32r) and vN [T,D] (fp32)
  - causal linear attention block-wise: energyT = qr@qr^T per
    [s128, t512] block (PSUM), bf16-mask multiply, then aN accumulated
    in PSUM over s-chunks
  - LayerNorms in natural layout with fused ACT Square/Identity
    (per-partition scale+bias + accum_out row sums)
  - MLP streamed over N in eighths (fp32 weights DMA'd per layer in
    host-pre-shuffled partition-contiguous layouts),
    relu(x)*relu(y) fused via scalar_tensor_tensor, update accumulated
    in PSUM then SBUF
  - PE 128x128 transposes maintain both v layouts
"""

import math

import numpy as np
import ml_dtypes

import concourse.bass as bass
import concourse.tile as tile
from concourse import bacc, mybir
from concourse import bass_utils

F32 = mybir.dt.float32
F32R = mybir.dt.float32r
BF16 = mybir.dt.bfloat16
I32 = mybir.dt.int32
ALU = mybir.AluOpType
ACTF = mybir.ActivationFunctionType
AXX = mybir.AxisListType.X

B, T, D, N, H, VOCAB, L = 8, 2048, 256, 8192, 4, 256, 6
EPS = 1e-5
TS = 512          # t-super width
NSUP = T // TS    # 4
NTB = T // 128    # 16
NQ = 8            # weight chunks along N
NCHQ = N // 128 // NQ  # 8 n-chunks per weight chunk


def build_nc(layers=L, stream_weights=True, attn=True, cphase=True):
    nc = bacc.Bacc("TRN2", target_bir_lowering=False, debug=False)

    idx_d = nc.dram_tensor("idxf", [1, T], F32R, kind="ExternalInput")
    wte_d = nc.dram_tensor("wte", [VOCAB, D], F32, kind="ExternalInput")
    wx_d = nc.dram_tensor("wx", [128, 2, N], F32R, kind="ExternalInput")
    wy_d = nc.dram_tensor("wy", [128, 2, N], F32R, kind="ExternalInput")
    enc_d = nc.dram_tensor("enc", [128, N // 128, D], F32R, kind="ExternalInput")
    ro_d = nc.dram_tensor("ro", [D, VOCAB], F32, kind="ExternalInput")
    cos_d = nc.dram_tensor("cosT", [128, T], F32, kind="ExternalInput")
    sin_d = nc.dram_tensor("sinT", [128, T], F32, kind="ExternalInput")
    mask_d = nc.dram_tensor("maskbig", [128, 1024], BF16, kind="ExternalInput")
    ident_d = nc.dram_tensor("identm", [128, 128], F32, kind="ExternalInput")
    out_d = nc.dram_tensor("logits", [T, VOCAB], F32, kind="ExternalOutput")

    wx_r, wy_r, enc_r = wx_d.ap(), wy_d.ap(), enc_d.ap()
    wte_r = wte_d.ap().rearrange("(c p) d -> p c d", p=128)
    ro_r = ro_d.ap().rearrange("(c p) d -> p c d", p=128)

    with tile.TileContext(nc) as tc:
        with tc.tile_pool(name="persist", bufs=1) as pp, \
             tc.tile_pool(name="wq", bufs=2) as wq, \
             tc.tile_pool(name="blk", bufs=8) as blkp, \
             tc.tile_pool(name="sc", bufs=18) as scp, \
             tc.tile_pool(name="st", bufs=48) as stp, \
             tc.tile_pool(name="ps512", bufs=4, space="PSUM") as ps512, \
             tc.tile_pool(name="ps256", bufs=4, space="PSUM") as ps256:

            vT = [pp.tile([128, T], F32R, name=f"vT{c}", tag=f"vT{c}") for c in range(2)]
            vN = pp.tile([128, NTB, D], F32, name="vN", tag="vN")
            vNr = pp.tile([128, NTB, D], F32R, name="vNr", tag="vNr")
            qrT = [pp.tile([128, T], F32R, name=f"qrT{c}", tag=f"qrT{c}") for c in range(2)]
            lnaT = [pp.tile([128, T], F32R, name=f"lnaT{c}", tag=f"lnaT{c}") for c in range(2)]
            updF = pp.tile([128, NTB * D], F32, name="updF", tag="updF")
            _updv = updF.rearrange("p (o d) -> p o d", d=D)

            def updA(tb):
                return _updv[:, tb, :]
            cosT = pp.tile([128, T], F32, name="cosT", tag="cosT")
            sinT = pp.tile([128, T], F32, name="sinT", tag="sinT")
            ropesc = pp.tile([128, TS], F32, name="ropesc", tag="ropesc")
            maskb = pp.tile([128, 1024], BF16, name="maskb", tag="maskb")

            ident = pp.tile([128, 128], F32, name="ident", tag="ident")
            iota_f = pp.tile([128, 2], F32, name="iota_f", tag="iota_f")

            nc.sync.dma_start(ident[:], ident_d.ap())
            nc.sync.dma_start(maskb[:], mask_d.ap())
            nc.sync.dma_start(cosT[:], cos_d.ap())
            nc.sync.dma_start(sinT[:], sin_d.ap())

            copy_flip = [0]

            def copy_any(dst, src):
                # alternate PSUM->SBUF copies between ACT and DVE
                copy_flip[0] ^= 1
                if copy_flip[0]:
                    nc.scalar.copy(dst, src)
                else:
                    nc.vector.tensor_copy(dst, src)

            def mm(psum, lhsT, rhs, start, stop):
                nc.tensor.matmul(psum, lhsT, rhs, start=start, stop=stop)

            def tr128(dst, src):
                pst = ps512.tile([128, 512], F32, name="pst", tag="ps512")
                nc.tensor.transpose(pst[:, :128], src, ident[:])
                copy_any(dst, pst[:, :128])

            def ln_nat_multi(items):
                """Batched LayerNorm over free dim (256): items is a list of
                (src, dst, sums_or_None). Emitted stage-wise across items so
                the per-item chains pipeline through the in-order ACT/DVE
                queues instead of serializing head-of-line."""
                n = len(items)
                sums_l, negmean_l, sqs_l, rstd_l, negmurs_l = [], [], [], [], []
                for src, dst, sums in items:
                    if sums is None:
                        sums = stp.tile([128, 1], F32, name="s1", tag="st")
                        nc.vector.reduce_sum(sums, src, axis=AXX)
                    sums_l.append(sums)
                for i in range(n):
                    negmean = stp.tile([128, 1], F32, name="negmean", tag="st")
                    nc.vector.tensor_scalar_mul(negmean, sums_l[i], -1.0 / D)
                    negmean_l.append(negmean)
                for i in range(n):
                    sq = scp.tile([128, D], F32, name="sq", tag="sc")
                    sqs = stp.tile([128, 1], F32, name="sqs", tag="st")
                    nc.scalar.activation(sq, items[i][0], ACTF.Square,
                                         bias=negmean_l[i], scale=1.0, accum_out=sqs)
                    sqs_l.append(sqs)
                veps_l = []
                for i in range(n):
                    veps = stp.tile([128, 1], F32, name="veps", tag="st")
                    nc.vector.tensor_scalar(veps, sqs_l[i], 1.0 / D, EPS,
                                            op0=ALU.mult, op1=ALU.add)
                    veps_l.append(veps)
                sqv_l = []
                for i in range(n):
                    sqv = stp.tile([128, 1], F32, name="sqv", tag="st")
                    nc.scalar.sqrt(sqv, veps_l[i])
                    sqv_l.append(sqv)
                for i in range(n):
                    rstd = stp.tile([128, 1], F32, name="rstd", tag="st")
                    nc.vector.reciprocal(rstd, sqv_l[i])
                    rstd_l.append(rstd)
                for i in range(n):
                    negmurs = stp.tile([128, 1], F32, name="negmurs", tag="st")
                    nc.vector.tensor_tensor(negmurs, negmean_l[i], rstd_l[i],
                                            op=ALU.mult)
                    negmurs_l.append(negmurs)
                for i in range(n):
                    nc.scalar.activation(items[i][1], items[i][0], ACTF.Identity,
                                         bias=negmurs_l[i], scale=rstd_l[i])

            def ln_nat(src, dst, sums=None):
                ln_nat_multi([(src, dst, sums)])

            def rope_si(si):
                # qrT[:, si block] = rope(vT[:, si block]); runs on the
                # otherwise-idle GPSIMD engine (SBUF-only operands)
                sl = slice(si * TS, (si + 1) * TS)
                nc.gpsimd.tensor_tensor(qrT[0][:, sl], vT[0][:, sl], cosT[:, sl],
                                        op=ALU.mult)
                nc.gpsimd.tensor_tensor(ropesc[:], vT[1][:, sl], sinT[:, sl],
                                        op=ALU.mult)
                nc.gpsimd.tensor_tensor(qrT[0][:, sl], qrT[0][:, sl], ropesc[:],
                                        op=ALU.subtract)
                nc.gpsimd.tensor_tensor(qrT[1][:, sl], vT[1][:, sl], cosT[:, sl],
                                        op=ALU.mult)
                nc.gpsimd.tensor_tensor(ropesc[:], vT[0][:, sl], sinT[:, sl],
                                        op=ALU.mult)
                nc.gpsimd.tensor_tensor(qrT[1][:, sl], qrT[1][:, sl], ropesc[:],
                                        op=ALU.add)

            # readout weights live in a persistent tile so the readout can
            # interleave with the last layer's cphase
            ro_s = pp.tile([128, 2, D], F32, name="ro_s", tag="ro_s")
            nc.sync.dma_start(ro_s[:], ro_r)

            def vmaint_si(si, layer):
                # PE-side per-si maintenance after cphase(si) wrote vN:
                # either rebuild vT (+ rope for the next layer), or run the
                # readout on the last layer (batched across the 4 tbs).
                tbs = list(range(si * 4, si * 4 + 4))
                if layer < layers - 1:
                    for tb in tbs:
                        for c in range(2):
                            tr128(vT[c][:, tb * 128:(tb + 1) * 128],
                                  vN[:, tb, c * 128:(c + 1) * 128])
                    rope_si(si)
                else:
                    vvs = {}
                    for tb in tbs:
                        vv = scp.tile([128, 2, 128], F32, name="vv", tag="sc")
                        for c in range(2):
                            tr128(vv[:, c, :], vN[:, tb, c * 128:(c + 1) * 128])
                        vvs[tb] = vv
                    los = {}
                    for tb in tbs:
                        psR = ps512.tile([128, 512], F32, name="psR", tag="ps512")
                        for c in range(2):
                            mm(psR[:, :D], vvs[tb][:, c, :], ro_s[:, c, :],
                               start=(c == 0), stop=(c == 1))
                        lo = scp.tile([128, VOCAB], F32, name="lo", tag="sc")
                        copy_any(lo[:], psR[:, :D])
                        los[tb] = lo
                    for tb in tbs:
                        nc.sync.dma_start(out_d.ap()[tb * 128:(tb + 1) * 128, :],
                                          los[tb][:])

            # ---------------- embedding: v = ln(wte[idx]) ----------------
            idx_b = lnaT[0]  # scratch alias
            nc.sync.dma_start(idx_b[:], idx_d.ap().partition_broadcast(128))
            wte_s = blkp.tile([128, 2, D], F32, name="wte_s", tag="blk")
            nc.sync.dma_start(wte_s[:], wte_r)
            iota_i = pp.tile([128, 2], I32, name="iota_i", tag="iota_i")
            for c in range(2):
                nc.gpsimd.iota(iota_i[:, c:c + 1], pattern=[[1, 1]], base=c * 128,
                               channel_multiplier=1)
            nc.vector.tensor_copy(iota_f[:], iota_i[:])
            onehot = [updF[:, 0:T], updF[:, T:2 * T]]  # scratch alias (F32)

            def embed_maint(si):
                for tb in range(si * 4, si * 4 + 4):
                    for c in range(2):
                        tr128(vT[c][:, tb * 128:(tb + 1) * 128],
                              vN[:, tb, c * 128:(c + 1) * 128])
                rope_si(si)

            for si in range(NSUP):
                sl = slice(si * TS, (si + 1) * TS)
                for c in range(2):
                    nc.vector.tensor_scalar(onehot[c][:, sl], idx_b[:, sl],
                                            iota_f[:, c:c + 1], None,
                                            op0=ALU.is_equal)
                psAs = {}
                for tb in range(si * 4, si * 4 + 4):
                    psA = ps256.tile([128, D], F32, name="psE", tag="ps256")
                    for c in range(2):
                        mm(psA, onehot[c][:, tb * 128:(tb + 1) * 128], wte_s[:, c, :],
                           start=(c == 0), stop=(c == 1))
                    psAs[tb] = psA
                ln_nat_multi([(psAs[tb], vN[:, tb, :], None)
                              for tb in range(si * 4, si * 4 + 4)])
                for tb in range(si * 4, si * 4 + 4):
                    nc.vector.tensor_copy(vNr[:, tb, :], vN[:, tb, :])
                if si >= 1:
                    embed_maint(si - 1)  # one si late: LN deps are ready
            embed_maint(NSUP - 1)

            # ---------------- layers ----------------
            if not stream_weights:
                wxq0 = wq.tile([128, 2, N // NQ], F32R, name="wxq", tag="wxq")
                nc.sync.dma_start(wxq0[:], wx_r[:, :, 0:N // NQ])
                wyq0 = wq.tile([128, 2, N // NQ], F32R, name="wyq", tag="wyq")
                nc.sync.dma_start(wyq0[:], wy_r[:, :, 0:N // NQ])
                encq0 = wq.tile([128, NCHQ, D], F32R, name="encq", tag="encq")
                nc.sync.dma_start(encq0[:], enc_r[:, 0:NCHQ, :])
            pend_vmaint_h = [None]
            for layer in range(layers):
                # (qrT for this layer was produced by the previous layer's
                #  interleaved rope_si calls, or by the embedding epilogue)
                pend_vmaint = pend_vmaint_h[0]
                pend_vmaint_h[0] = None

                # --- attention + LN(a) -> lnaT ---
                # psE runs one sc ahead of psA (skew-1) so PE covers the
                # DVE masking latency; LN(a) transposes land one si late so
                # their LN-chain deps are satisfied; the previous layer's
                # si=3 vT/rope maintenance is emitted after si=0's matmuls.
                lna_pend = {}

                def emit_energy(si, sc, eTs):
                    psE = ps512.tile([128, TS], F32, name="psE", tag="ps512")
                    for c in range(2):
                        mm(psE, qrT[c][:, sc * 128:(sc + 1) * 128],
                           qrT[c][:, si * TS:(si + 1) * TS],
                           start=(c == 0), stop=(c == 1))
                    eT = blkp.tile([128, TS], F32R, name="eT", tag="blk")
                    k = sc - 4 * si
                    if k < 0:
                        copy_any(eT[:], psE[:])
                    else:
                        nc.vector.tensor_tensor(
                            eT[:], psE[:], maskb[:, 384 - k * 128: 896 - k * 128],
                            op=ALU.mult)
                    eTs[sc] = eT

                def emit_a(si, sc, eTs, psA):
                    eT = eTs.pop(sc)
                    for tb4 in range(4):
                        tb = si * 4 + tb4
                        if sc <= tb:
                            mm(psA[tb4], eT[:, tb4 * 128:(tb4 + 1) * 128],
                               vNr[:, sc, :], start=(sc == 0), stop=(sc == tb))

                def emit_lna_tr(si):
                    for tb4 in range(4):
                        tb = si * 4 + tb4
                        lna_n = lna_pend.pop((si, tb4))
                        for c in range(2):
                            tr128(lnaT[c][:, tb * 128:(tb + 1) * 128],
                                  lna_n[:, c * 128:(c + 1) * 128])

                for si in range(NSUP if attn else 0):
                    psA = [ps256.tile([128, D], F32, name="psA", tag="ps256")
                           for _ in range(4)]
                    nsc = 4 * si + 4
                    eTs = {}
                    emit_energy(si, 0, eTs)
                    emit_energy(si, 1, eTs)
                    for sc in range(2, nsc):
                        emit_energy(si, sc, eTs)
                        emit_a(si, sc - 2, eTs, psA)
                    emit_a(si, nsc - 2, eTs, psA)
                    emit_a(si, nsc - 1, eTs, psA)
                    lna_items = []
                    for tb4 in range(4):
                        lna_n = scp.tile([128, D], F32, name="lna_n", tag="sc")
                        lna_items.append((psA[tb4], lna_n, None))
                        lna_pend[(si, tb4)] = lna_n
                    ln_nat_multi(lna_items)
                    if si == 1 and pend_vmaint is not None:
                        pend_vmaint()
                    if si >= 1:
                        emit_lna_tr(si - 1)
                if attn:
                    emit_lna_tr(NSUP - 1)

                # --- MLP over N eighths: one flat skew-1 pipeline over
                # (q, si, nch) so the in-order PE stream never drains at
                # (q, si) boundaries ---
                upd_sums = {}
                ln_src = lnaT if attn else qrT
                w_cache = {}

                def fetch_w(q):
                    if q in w_cache:
                        return w_cache[q]
                    if stream_weights:
                        qs = slice(q * (N // NQ), (q + 1) * (N // NQ))
                        wxq = wq.tile([128, 2, N // NQ], F32R, name="wxq", tag="wxq")
                        nc.sync.dma_start(wxq[:], wx_r[:, :, qs])
                        wyq = wq.tile([128, 2, N // NQ], F32R, name="wyq", tag="wyq")
                        nc.sync.dma_start(wyq[:], wy_r[:, :, qs])
                        encq = wq.tile([128, NCHQ, D], F32R, name="encq", tag="encq")
                        nc.sync.dma_start(encq[:], enc_r[:, q * NCHQ:(q + 1) * NCHQ, :])
                        w_cache[q] = (wxq, wyq, encq)
                    else:
                        w_cache[q] = (wxq0, wyq0, encq0)
                    return w_cache[q]

                psU_cache = {}

                def get_psU(q, si):
                    if (q, si) not in psU_cache:
                        psU_cache[(q, si)] = [
                            ps256.tile([128, D], F32, name="psU", tag="ps256")
                            for _ in range(4)]
                    return psU_cache[(q, si)]

                ys = {}

                def emit_xy(q, si, nch):
                    wxq, wyq, encq = fetch_w(q)
                    sl = slice(si * TS, (si + 1) * TS)
                    psX = ps512.tile([128, TS], F32, name="psX", tag="ps512")
                    psY = ps512.tile([128, TS], F32, name="psY", tag="ps512")
                    ns = slice(nch * 128, (nch + 1) * 128)
                    for c in range(2):
                        mm(psX, wxq[:, c, ns], vT[c][:, sl],
                           start=(c == 0), stop=(c == 1))
                        mm(psY, wyq[:, c, ns], ln_src[c][:, sl],
                           start=(c == 0), stop=(c == 1))
                    xr = blkp.tile([128, TS], F32, name="xr", tag="blk")
                    nc.scalar.activation(xr, psX, ACTF.Relu)
                    ysb = blkp.tile([128, TS], F32R, name="ysb", tag="blk")
                    nc.vector.scalar_tensor_tensor(
                        ysb, psY, 0.0, xr, op0=ALU.max, op1=ALU.mult)
                    ys[(q, si, nch)] = ysb

                def emit_adds(q, si):
                    psU = psU_cache.pop((q, si))
                    for tb4 in range(4):
                        tb = si * 4 + tb4
                        dst = updA(tb)
                        if q == 0:
                            nc.scalar.copy(dst, psU[tb4])
                        elif q < NQ - 1:
                            nc.vector.tensor_tensor(dst, psU[tb4], dst, op=ALU.add)
                        else:
                            s2 = stp.tile([128, 1], F32, name="s2", tag="st")
                            nc.vector.scalar_tensor_tensor(
                                dst, psU[tb4], 0.0, dst, op0=ALU.add,
                                op1=ALU.add, accum_out=s2)
                            upd_sums[tb] = s2

                def emit_cphase(si):
                    # interleaved cphase for this si (DVE/ACT only),
                    # stage-batched across the 4 tbs:
                    # v = ln(v + ln(update)) -> vN, vNr
                    tbs = list(range(si * 4, si * 4 + 4))
                    lnus = {}
                    for tb in tbs:
                        lnus[tb] = scp.tile([128, D], F32, name="lnu", tag="sc")
                    ln_nat_multi([(updA(tb), lnus[tb], upd_sums[tb])
                                  for tb in tbs])
                    vmids, s3s = {}, {}
                    for tb in tbs:
                        vmid = scp.tile([128, D], F32, name="vmid", tag="sc")
                        s3 = stp.tile([128, 1], F32, name="s3", tag="st")
                        nc.vector.scalar_tensor_tensor(
                            vmid, lnus[tb], 0.0, vN[:, tb, :], op0=ALU.add,
                            op1=ALU.add, accum_out=s3)
                        vmids[tb], s3s[tb] = vmid, s3
                    ln_nat_multi([(vmids[tb], vN[:, tb, :], s3s[tb])
                                  for tb in tbs])
                    if layer < layers - 1:
                        for tb in tbs:
                            nc.gpsimd.tensor_copy(vNr[:, tb, :], vN[:, tb, :])

                def emit_u(q, si, nch):
                    _, _, encq = fetch_w(q)
                    psU = get_psU(q, si)
                    ysb = ys.pop((q, si, nch))
                    for tb4 in range(4):
                        t4 = slice(tb4 * 128, (tb4 + 1) * 128)
                        mm(psU[tb4], ysb[:, t4], encq[:, nch, :],
                           start=(nch == 0), stop=(nch == NCHQ - 1))
                    if nch == NCHQ - 1:
                        emit_adds(q, si)
                        if q == NQ - 1 and cphase:
                            emit_cphase(si)
                            if si >= 1:
                                vmaint_si(si - 1, layer)
                            if si == NSUP - 1:
                                if layer < layers - 1:
                                    pend_vmaint_h[0] = (
                                        lambda si=si, layer=layer:
                                        vmaint_si(si, layer))
                                else:
                                    vmaint_si(si, layer)  # final readout tail

                prev = None
                for q in range(NQ):
                    for si in range(NSUP):
                        for nch in range(NCHQ):
                            emit_xy(q, si, nch)
                            if prev is not None:
                                emit_u(*prev)
                            prev = (q, si, nch)
                emit_u(*prev)

    nc.compile()
    return nc


_NC_CACHE = {}


def get_nc():
    if "nc" not in _NC_CACHE:
        _NC_CACHE["nc"] = build_nc()
    return _NC_CACHE["nc"]


def make_host_inputs(idx, wte, encoder, decoder_x, decoder_y, readout):
    idx = np.asarray(idx)
    wte = np.asarray(wte, dtype=np.float32)
    encoder = np.asarray(encoder, dtype=np.float32)
    decoder_x = np.asarray(decoder_x, dtype=np.float32)
    decoder_y = np.asarray(decoder_y, dtype=np.float32)
    readout = np.asarray(readout, dtype=np.float32)

    wx = decoder_x.transpose(1, 0, 2).reshape(D, N)
    wy = decoder_y.transpose(1, 0, 2).reshape(D, N)
    # partition-contiguous layouts for fast DMA: [p, c, n] with d = c*128 + p
    wx = np.ascontiguousarray(wx.reshape(2, 128, N).transpose(1, 0, 2))
    wy = np.ascontiguousarray(wy.reshape(2, 128, N).transpose(1, 0, 2))
    # enc: [p, o, d] with n = o*128 + p
    enc_s = np.ascontiguousarray(encoder.reshape(N // 128, 128, D).transpose(1, 0, 2))

    inv_freq = 1.0 / (10000.0 ** (np.arange(0, D, 2, dtype=np.float32) / D))  # [128]
    t = np.arange(T, dtype=np.float32)
    freqsT = inv_freq[:, None] * t[None, :]                   # [128, T]
    cosT = np.cos(freqsT).astype(np.float32)
    sinT = np.sin(freqsT).astype(np.float32)

    s_idx = np.arange(128, dtype=np.int32)[:, None]
    c_idx = np.arange(1024, dtype=np.int32)[None, :]
    maskbig = (s_idx <= c_idx - 384).astype(ml_dtypes.bfloat16)

    in_maps = []
    for b in range(B):
        in_maps.append({
            "idxf": idx[b].astype(np.float32).reshape(1, T),
            "wte": wte,
            "wx": wx,
            "wy": wy,
            "enc": enc_s,
            "ro": readout,
            "cosT": cosT,
            "sinT": sinT,
            "maskbig": maskbig,
            "identm": np.eye(128, dtype=np.float32),
        })
    return in_maps


def kernel(idx, wte, encoder, decoder_x, decoder_y, readout):
    nc = get_nc()
    in_maps = make_host_inputs(idx, wte, encoder, decoder_x, decoder_y, readout)
    res = bass_utils.run_bass_kernel_spmd(nc, in_maps, core_ids=list(range(B)))
    out = np.stack([res.results[b]["logits"] for b in range(B)], axis=0)
    return out.astype(np.float32)


# revision 64
# speedup vs baseline: 1.0531x; 1.0092x over previous
"""Trainium2 Bass kernel for the BDH dense-transformer problem.

Sharding: data-parallel over B=8 across the 8 NeuronCores (one batch
element per core, no collectives). Each core runs the full 6-layer
network on its [T=2048, D=256] slice.

Per-core program. Matmul precision strategy:
  - the per-layer FLOP bulk (attention energy/a, MLP x/y/update) runs
    in float32r: 1 cyc/row on the PE when the output free dim is
    >= 256 (true for all matmuls here), vs 4 cyc/row for fp32 and
    3 cyc/row for the bf16x2 3-pass split scheme. No host splits and
    no DVE split work. All tensors feeding an f32r matmul are declared
    float32r so writes round appropriately (BIR verifier requirement).
  - precision recovery: the residual stream vN and the update
    accumulator updW stay full fp32 (vNr is a rounded F32R mirror used
    only as the attention a-matmul input), and the run-once embedding
    and readout matmuls are full fp32. Only per-layer matmul-input
    roundings remain.
Structure:
  - token embedding via one-hot matmul (iota + is_equal + PE)
  - v kept in both layouts: vT [D,T] (f32r) and vN [T,D] (fp32)
  - causal linear attention block-wise: energyT = qr@qr^T per
    [s128, t512] block (PSUM), bf16-mask multiply, then aN accumulated
    in PSUM over s-chunks
  - LayerNorms in natural layout with fused ACT Square/Identity
    (per-partition scale+bias + accum_out row sums)
  - MLP streamed over N in eighths (fp32 weights DMA'd per layer in
    host-pre-shuffled partition-contiguous layouts),
    relu(x)*relu(y) fused via scalar_tensor_tensor, update accumulated
    in PSUM then SBUF
  - PE 128x128 transposes maintain both v layouts
"""

import math

import numpy as np
import ml_dtypes

import concourse.bass as bass
import concourse.tile as tile
from concourse import bacc, mybir
from concourse import bass_utils

F32 = mybir.dt.float32
F32R = mybir.dt.float32r
BF16 = mybir.dt.bfloat16
I32 = mybir.dt.int32
ALU = mybir.AluOpType
ACTF = mybir.ActivationFunctionType
AXX = mybir.AxisListType.X

B, T, D, N, H, VOCAB, L = 8, 2048, 256, 8192, 4, 256, 6
EPS = 1e-5
TS = 512          # t-super width
NSUP = T // TS    # 4
NTB = T // 128    # 16
NQ = 8            # weight chunks along N
NCHQ = N // 128 // NQ  # 8 n-chunks per weight chunk


def build_nc(layers=L, stream_weights=True, attn=True, cphase=True):
    nc = bacc.Bacc("TRN2", target_bir_lowering=False, debug=False)

    idx_d = nc.dram_tensor("idxf", [1, T], F32R, kind="ExternalInput")
    wte_d = nc.dram_tensor("wte", [VOCAB, D], F32, kind="ExternalInput")
    wx_d = nc.dram_tensor("wx", [128, 2, N], F32R, kind="ExternalInput")
    wy_d = nc.dram_tensor("wy", [128, 2, N], F32R, kind="ExternalInput")
    enc_d = nc.dram_tensor("enc", [128, N // 128, D], F32R, kind="ExternalInput")
    ro_d = nc.dram_tensor("ro", [D, VOCAB], F32, kind="ExternalInput")
    cos_d = nc.dram_tensor("cosT", [128, T], F32, kind="ExternalInput")
    sin_d = nc.dram_tensor("sinT", [128, T], F32, kind="ExternalInput")
    mask_d = nc.dram_tensor("maskbig", [128, 1024], BF16, kind="ExternalInput")
    ident_d = nc.dram_tensor("identm", [128, 128], F32, kind="ExternalInput")
    out_d = nc.dram_tensor("logits", [T, VOCAB], F32, kind="ExternalOutput")

    wx_r, wy_r, enc_r = wx_d.ap(), wy_d.ap(), enc_d.ap()
    wte_r = wte_d.ap().rearrange("(c p) d -> p c d", p=128)
    ro_r = ro_d.ap().rearrange("(c p) d -> p c d", p=128)

    with tile.TileContext(nc) as tc:
        with tc.tile_pool(name="persist", bufs=1) as pp, \
             tc.tile_pool(name="wq", bufs=2) as wq, \
             tc.tile_pool(name="blk", bufs=8) as blkp, \
             tc.tile_pool(name="sc", bufs=18) as scp, \
             tc.tile_pool(name="st", bufs=48) as stp, \
             tc.tile_pool(name="ps512", bufs=4, space="PSUM") as ps512, \
             tc.tile_pool(name="ps256", bufs=4, space="PSUM") as ps256:

            vT = [pp.tile([128, T], F32R, name=f"vT{c}", tag=f"vT{c}") for c in range(2)]
            vN = pp.tile([128, NTB, D], F32, name="vN", tag="vN")
            vNr = pp.tile([128, NTB, D], F32R, name="vNr", tag="vNr")
            qrT = [pp.tile([128, T], F32R, name=f"qrT{c}", tag=f"qrT{c}") for c in range(2)]
            lnaT = [pp.tile([128, T], F32R, name=f"lnaT{c}", tag=f"lnaT{c}") for c in range(2)]
            updF = pp.tile([128, NTB * D], F32, name="updF", tag="updF")
            _updv = updF.rearrange("p (o d) -> p o d", d=D)

            def updA(tb):
                return _updv[:, tb, :]
            cosT = pp.tile([128, T], F32, name="cosT", tag="cosT")
            sinT = pp.tile([128, T], F32, name="sinT", tag="sinT")
            ropesc = pp.tile([128, TS], F32, name="ropesc", tag="ropesc")
            maskb = pp.tile([128, 1024], BF16, name="maskb", tag="maskb")

            ident = pp.tile([128, 128], F32, name="ident", tag="ident")
            iota_f = pp.tile([128, 2], F32, name="iota_f", tag="iota_f")

            nc.sync.dma_start(ident[:], ident_d.ap())
            nc.sync.dma_start(maskb[:], mask_d.ap())
            nc.sync.dma_start(cosT[:], cos_d.ap())
            nc.sync.dma_start(sinT[:], sin_d.ap())

            copy_flip = [0]

            def copy_any(dst, src):
                # alternate PSUM->SBUF copies between ACT and DVE
                copy_flip[0] ^= 1
                if copy_flip[0]:
                    nc.scalar.copy(dst, src)
                else:
                    nc.vector.tensor_copy(dst, src)

            def mm(psum, lhsT, rhs, start, stop):
                nc.tensor.matmul(psum, lhsT, rhs, start=start, stop=stop)

            def tr128(dst, src):
                pst = ps512.tile([128, 512], F32, name="pst", tag="ps512")
                nc.tensor.transpose(pst[:, :128], src, ident[:])
                copy_any(dst, pst[:, :128])

            def ln_nat_multi(items, zero_mean=False):
                """Batched LayerNorm over free dim (256): items is a list of
                (src, dst, sums_or_None). Emitted stage-wise across items so
                the per-item chains pipeline through the in-order ACT/DVE
                queues instead of serializing head-of-line.

                zero_mean=True: the rows are exact linear combinations of
                prior LayerNorm outputs, so their mean is identically zero;
                skip the mean subtraction entirely."""
                n = len(items)
                sums_l, negmean_l, sqs_l, rstd_l, negmurs_l = [], [], [], [], []
                if not zero_mean:
                    for src, dst, sums in items:
                        if sums is None:
                            sums = stp.tile([128, 1], F32, name="s1", tag="st")
                            nc.vector.reduce_sum(sums, src, axis=AXX)
                        sums_l.append(sums)
                    for i in range(n):
                        negmean = stp.tile([128, 1], F32, name="negmean", tag="st")
                        nc.vector.tensor_scalar_mul(negmean, sums_l[i], -1.0 / D)
                        negmean_l.append(negmean)
                for i in range(n):
                    sq = scp.tile([128, D], F32, name="sq", tag="sc")
                    sqs = stp.tile([128, 1], F32, name="sqs", tag="st")
                    nc.scalar.activation(sq, items[i][0], ACTF.Square,
                                         bias=(0.0 if zero_mean else negmean_l[i]),
                                         scale=1.0, accum_out=sqs)
                    sqs_l.append(sqs)
                veps_l = []
                for i in range(n):
                    veps = stp.tile([128, 1], F32, name="veps", tag="st")
                    nc.vector.tensor_scalar(veps, sqs_l[i], 1.0 / D, EPS,
                                            op0=ALU.mult, op1=ALU.add)
                    veps_l.append(veps)
                sqv_l = []
                for i in range(n):
                    sqv = stp.tile([128, 1], F32, name="sqv", tag="st")
                    nc.scalar.sqrt(sqv, veps_l[i])
                    sqv_l.append(sqv)
                for i in range(n):
                    rstd = stp.tile([128, 1], F32, name="rstd", tag="st")
                    nc.vector.reciprocal(rstd, sqv_l[i])
                    rstd_l.append(rstd)
                if not zero_mean:
                    for i in range(n):
                        negmurs = stp.tile([128, 1], F32, name="negmurs", tag="st")
                        nc.vector.tensor_tensor(negmurs, negmean_l[i], rstd_l[i],
                                                op=ALU.mult)
                        negmurs_l.append(negmurs)
                for i in range(n):
                    nc.scalar.activation(items[i][1], items[i][0], ACTF.Identity,
                                         bias=(0.0 if zero_mean else negmurs_l[i]),
                                         scale=rstd_l[i])

            def ln_nat(src, dst, sums=None):
                ln_nat_multi([(src, dst, sums)])

            def rope_si(si):
                # qrT[:, si block] = rope(vT[:, si block]); runs on the
                # otherwise-idle GPSIMD engine (SBUF-only operands)
                sl = slice(si * TS, (si + 1) * TS)
                nc.gpsimd.tensor_tensor(qrT[0][:, sl], vT[0][:, sl], cosT[:, sl],
                                        op=ALU.mult)
                nc.gpsimd.tensor_tensor(ropesc[:], vT[1][:, sl], sinT[:, sl],
                                        op=ALU.mult)
                nc.gpsimd.tensor_tensor(qrT[0][:, sl], qrT[0][:, sl], ropesc[:],
                                        op=ALU.subtract)
                nc.gpsimd.tensor_tensor(qrT[1][:, sl], vT[1][:, sl], cosT[:, sl],
                                        op=ALU.mult)
                nc.gpsimd.tensor_tensor(ropesc[:], vT[0][:, sl], sinT[:, sl],
                                        op=ALU.mult)
                nc.gpsimd.tensor_tensor(qrT[1][:, sl], qrT[1][:, sl], ropesc[:],
                                        op=ALU.add)

            # readout weights live in a persistent tile so the readout can
            # interleave with the last layer's cphase
            ro_s = pp.tile([128, 2, D], F32, name="ro_s", tag="ro_s")
            nc.sync.dma_start(ro_s[:], ro_r)

            def vmaint_si(si, layer):
                # PE-side per-si maintenance after cphase(si) wrote vN:
                # either rebuild vT (+ rope for the next layer), or run the
                # readout on the last layer (batched across the 4 tbs).
                tbs = list(range(si * 4, si * 4 + 4))
                if layer < layers - 1:
                    for tb in tbs:
                        for c in range(2):
                            tr128(vT[c][:, tb * 128:(tb + 1) * 128],
                                  vN[:, tb, c * 128:(c + 1) * 128])
                    rope_si(si)
                else:
                    vvs = {}
                    for tb in tbs:
                        vv = scp.tile([128, 2, 128], F32, name="vv", tag="sc")
                        for c in range(2):
                            tr128(vv[:, c, :], vN[:, tb, c * 128:(c + 1) * 128])
                        vvs[tb] = vv
                    los = {}
                    for tb in tbs:
                        psR = ps512.tile([128, 512], F32, name="psR", tag="ps512")
                        for c in range(2):
                            mm(psR[:, :D], vvs[tb][:, c, :], ro_s[:, c, :],
                               start=(c == 0), stop=(c == 1))
                        lo = scp.tile([128, VOCAB], F32, name="lo", tag="sc")
                        copy_any(lo[:], psR[:, :D])
                        los[tb] = lo
                    for tb in tbs:
                        nc.sync.dma_start(out_d.ap()[tb * 128:(tb + 1) * 128, :],
                                          los[tb][:])

            # ---------------- embedding: v = ln(wte[idx]) ----------------
            idx_b = lnaT[0]  # scratch alias
            nc.sync.dma_start(idx_b[:], idx_d.ap().partition_broadcast(128))
            wte_s = blkp.tile([128, 2, D], F32, name="wte_s", tag="blk")
            nc.sync.dma_start(wte_s[:], wte_r)
            iota_i = pp.tile([128, 2], I32, name="iota_i", tag="iota_i")
            for c in range(2):
                nc.gpsimd.iota(iota_i[:, c:c + 1], pattern=[[1, 1]], base=c * 128,
                               channel_multiplier=1)
            nc.vector.tensor_copy(iota_f[:], iota_i[:])
            onehot = [updF[:, 0:T], updF[:, T:2 * T]]  # scratch alias (F32)

            def embed_maint(si):
                for tb in range(si * 4, si * 4 + 4):
                    for c in range(2):
                        tr128(vT[c][:, tb * 128:(tb + 1) * 128],
                              vN[:, tb, c * 128:(c + 1) * 128])
                rope_si(si)

            for si in range(NSUP):
                sl = slice(si * TS, (si + 1) * TS)
                for c in range(2):
                    nc.vector.tensor_scalar(onehot[c][:, sl], idx_b[:, sl],
                                            iota_f[:, c:c + 1], None,
                                            op0=ALU.is_equal)
                psAs = {}
                for tb in range(si * 4, si * 4 + 4):
                    psA = ps256.tile([128, D], F32, name="psE", tag="ps256")
                    for c in range(2):
                        mm(psA, onehot[c][:, tb * 128:(tb + 1) * 128], wte_s[:, c, :],
                           start=(c == 0), stop=(c == 1))
                    psAs[tb] = psA
                ln_nat_multi([(psAs[tb], vN[:, tb, :], None)
                              for tb in range(si * 4, si * 4 + 4)])
                for tb in range(si * 4, si * 4 + 4):
                    nc.vector.tensor_copy(vNr[:, tb, :], vN[:, tb, :])
                if si >= 1:
                    embed_maint(si - 1)  # one si late: LN deps are ready
            embed_maint(NSUP - 1)

            # ---------------- layers ----------------
            if not stream_weights:
                wxq0 = wq.tile([128, 2, N // NQ], F32R, name="wxq", tag="wxq")
                nc.sync.dma_start(wxq0[:], wx_r[:, :, 0:N // NQ])
                wyq0 = wq.tile([128, 2, N // NQ], F32R, name="wyq", tag="wyq")
                nc.sync.dma_start(wyq0[:], wy_r[:, :, 0:N // NQ])
                encq0 = wq.tile([128, NCHQ, D], F32R, name="encq", tag="encq")
                nc.sync.dma_start(encq0[:], enc_r[:, 0:NCHQ, :])
            pend_vmaint_h = [None]
            for layer in range(layers):
                # (qrT for this layer was produced by the previous layer's
                #  interleaved rope_si calls, or by the embedding epilogue)
                pend_vmaint = pend_vmaint_h[0]
                pend_vmaint_h[0] = None

                # --- attention + LN(a) -> lnaT ---
                # psE runs one sc ahead of psA (skew-1) so PE covers the
                # DVE masking latency; LN(a) transposes land one si late so
                # their LN-chain deps are satisfied; the previous layer's
                # si=3 vT/rope maintenance is emitted after si=0's matmuls.
                lna_pend = {}

                def emit_energy(si, sc, eTs):
                    psE = ps512.tile([128, TS], F32, name="psE", tag="ps512")
                    for c in range(2):
                        mm(psE, qrT[c][:, sc * 128:(sc + 1) * 128],
                           qrT[c][:, si * TS:(si + 1) * TS],
                           start=(c == 0), stop=(c == 1))
                    eT = blkp.tile([128, TS], F32R, name="eT", tag="blk")
                    k = sc - 4 * si
                    if k < 0:
                        copy_any(eT[:], psE[:])
                    else:
                        nc.vector.tensor_tensor(
                            eT[:], psE[:], maskb[:, 384 - k * 128: 896 - k * 128],
                            op=ALU.mult)
                    eTs[sc] = eT

                def emit_a(si, sc, eTs, psA):
                    eT = eTs.pop(sc)
                    for tb4 in range(4):
                        tb = si * 4 + tb4
                        if sc <= tb:
                            mm(psA[tb4], eT[:, tb4 * 128:(tb4 + 1) * 128],
                               vNr[:, sc, :], start=(sc == 0), stop=(sc == tb))

                def emit_lna_tr(si):
                    for tb4 in range(4):
                        tb = si * 4 + tb4
                        lna_n = lna_pend.pop((si, tb4))
                        for c in range(2):
                            tr128(lnaT[c][:, tb * 128:(tb + 1) * 128],
                                  lna_n[:, c * 128:(c + 1) * 128])

                for si in range(NSUP if attn else 0):
                    psA = [ps256.tile([128, D], F32, name="psA", tag="ps256")
                           for _ in range(4)]
                    nsc = 4 * si + 4
                    eTs = {}
                    emit_energy(si, 0, eTs)
                    emit_energy(si, 1, eTs)
                    for sc in range(2, nsc):
                        emit_energy(si, sc, eTs)
                        emit_a(si, sc - 2, eTs, psA)
                    emit_a(si, nsc - 2, eTs, psA)
                    emit_a(si, nsc - 1, eTs, psA)
                    lna_items = []
                    for tb4 in range(4):
                        lna_n = scp.tile([128, D], F32, name="lna_n", tag="sc")
                        lna_items.append((psA[tb4], lna_n, None))
                        lna_pend[(si, tb4)] = lna_n
                    # a's rows are linear combos of zero-mean v rows
                    ln_nat_multi(lna_items, zero_mean=True)
                    if si == 1 and pend_vmaint is not None:
                        pend_vmaint()
                    if si >= 1:
                        emit_lna_tr(si - 1)
                if attn:
                    emit_lna_tr(NSUP - 1)

                # --- MLP over N eighths: one flat skew-1 pipeline over
                # (q, si, nch) so the in-order PE stream never drains at
                # (q, si) boundaries ---
                upd_sums = {}
                ln_src = lnaT if attn else qrT
                w_cache = {}

                def fetch_w(q):
                    if q in w_cache:
                        return w_cache[q]
                    if stream_weights:
                        qs = slice(q * (N // NQ), (q + 1) * (N // NQ))
                        wxq = wq.tile([128, 2, N // NQ], F32R, name="wxq", tag="wxq")
                        nc.sync.dma_start(wxq[:], wx_r[:, :, qs])
                        wyq = wq.tile([128, 2, N // NQ], F32R, name="wyq", tag="wyq")
                        nc.sync.dma_start(wyq[:], wy_r[:, :, qs])
                        encq = wq.tile([128, NCHQ, D], F32R, name="encq", tag="encq")
                        nc.sync.dma_start(encq[:], enc_r[:, q * NCHQ:(q + 1) * NCHQ, :])
                        w_cache[q] = (wxq, wyq, encq)
                    else:
                        w_cache[q] = (wxq0, wyq0, encq0)
                    return w_cache[q]

                psU_cache = {}

                def get_psU(q, si):
                    if (q, si) not in psU_cache:
                        psU_cache[(q, si)] = [
                            ps256.tile([128, D], F32, name="psU", tag="ps256")
                            for _ in range(4)]
                    return psU_cache[(q, si)]

                ys = {}

                def emit_xy(q, si, nch):
                    wxq, wyq, encq = fetch_w(q)
                    sl = slice(si * TS, (si + 1) * TS)
                    psX = ps512.tile([128, TS], F32, name="psX", tag="ps512")
                    psY = ps512.tile([128, TS], F32, name="psY", tag="ps512")
                    ns = slice(nch * 128, (nch + 1) * 128)
                    for c in range(2):
                        mm(psX, wxq[:, c, ns], vT[c][:, sl],
                           start=(c == 0), stop=(c == 1))
                        mm(psY, wyq[:, c, ns], ln_src[c][:, sl],
                           start=(c == 0), stop=(c == 1))
                    xr = blkp.tile([128, TS], F32, name="xr", tag="blk")
                    nc.scalar.activation(xr, psX, ACTF.Relu)
                    ysb = blkp.tile([128, TS], F32R, name="ysb", tag="blk")
                    nc.vector.scalar_tensor_tensor(
                        ysb, psY, 0.0, xr, op0=ALU.max, op1=ALU.mult)
                    ys[(q, si, nch)] = ysb

                def emit_adds(q, si):
                    psU = psU_cache.pop((q, si))
                    for tb4 in range(4):
                        tb = si * 4 + tb4
                        dst = updA(tb)
                        if q == 0:
                            nc.scalar.copy(dst, psU[tb4])
                        elif q < NQ - 1:
                            nc.vector.tensor_tensor(dst, psU[tb4], dst, op=ALU.add)
                        else:
                            s2 = stp.tile([128, 1], F32, name="s2", tag="st")
                            nc.vector.scalar_tensor_tensor(
                                dst, psU[tb4], 0.0, dst, op0=ALU.add,
                                op1=ALU.add, accum_out=s2)
                            upd_sums[tb] = s2

                def emit_cphase(si):
                    # interleaved cphase for this si (DVE/ACT only),
                    # stage-batched across the 4 tbs:
                    # v = ln(v + ln(update)) -> vN, vNr
                    tbs = list(range(si * 4, si * 4 + 4))
                    lnus = {}
                    for tb in tbs:
                        lnus[tb] = scp.tile([128, D], F32, name="lnu", tag="sc")
                    ln_nat_multi([(updA(tb), lnus[tb], upd_sums[tb])
                                  for tb in tbs])
                    vmids = {}
                    for tb in tbs:
                        vmid = scp.tile([128, D], F32, name="vmid", tag="sc")
                        nc.vector.scalar_tensor_tensor(
                            vmid, lnus[tb], 0.0, vN[:, tb, :], op0=ALU.add,
                            op1=ALU.add)
                        vmids[tb] = vmid
                    # lnu and vN are both LayerNorm outputs: zero-mean sum
                    ln_nat_multi([(vmids[tb], vN[:, tb, :], None)
                                  for tb in tbs], zero_mean=True)
                    if layer < layers - 1:
                        for tb in tbs:
                            nc.gpsimd.tensor_copy(vNr[:, tb, :], vN[:, tb, :])

                def emit_u(q, si, nch):
                    _, _, encq = fetch_w(q)
                    psU = get_psU(q, si)
                    ysb = ys.pop((q, si, nch))
                    for tb4 in range(4):
                        t4 = slice(tb4 * 128, (tb4 + 1) * 128)
                        mm(psU[tb4], ysb[:, t4], encq[:, nch, :],
                           start=(nch == 0), stop=(nch == NCHQ - 1))
                    if nch == NCHQ - 1:
                        emit_adds(q, si)
                        if q == NQ - 1 and cphase:
                            emit_cphase(si)
                            if si >= 1:
                                vmaint_si(si - 1, layer)
                            if si == NSUP - 1:
                                if layer < layers - 1:
                                    pend_vmaint_h[0] = (
                                        lambda si=si, layer=layer:
                                        vmaint_si(si, layer))
                                else:
                                    vmaint_si(si, layer)  # final readout tail

                prev = None
                for q in range(NQ):
                    for si in range(NSUP):
                        for nch in range(NCHQ):
                            emit_xy(q, si, nch)
                            if prev is not None:
                                emit_u(*prev)
                            prev = (q, si, nch)
                emit_u(*prev)

    nc.compile()
    return nc


_NC_CACHE = {}


def get_nc():
    if "nc" not in _NC_CACHE:
        _NC_CACHE["nc"] = build_nc()
    return _NC_CACHE["nc"]


def make_host_inputs(idx, wte, encoder, decoder_x, decoder_y, readout):
    idx = np.asarray(idx)
    wte = np.asarray(wte, dtype=np.float32)
    encoder = np.asarray(encoder, dtype=np.float32)
    decoder_x = np.asarray(decoder_x, dtype=np.float32)
    decoder_y = np.asarray(decoder_y, dtype=np.float32)
    readout = np.asarray(readout, dtype=np.float32)

    wx = decoder_x.transpose(1, 0, 2).reshape(D, N)
    wy = decoder_y.transpose(1, 0, 2).reshape(D, N)
    # partition-contiguous layouts for fast DMA: [p, c, n] with d = c*128 + p
    wx = np.ascontiguousarray(wx.reshape(2, 128, N).transpose(1, 0, 2))
    wy = np.ascontiguousarray(wy.reshape(2, 128, N).transpose(1, 0, 2))
    # enc: [p, o, d] with n = o*128 + p
    enc_s = np.ascontiguousarray(encoder.reshape(N // 128, 128, D).transpose(1, 0, 2))

    inv_freq = 1.0 / (10000.0 ** (np.arange(0, D, 2, dtype=np.float32) / D))  # [128]
    t = np.arange(T, dtype=np.float32)
    freqsT = inv_freq[:, None] * t[None, :]                   # [128, T]
    cosT = np.cos(freqsT).astype(np.float32)
    sinT = np.sin(freqsT).astype(np.float32)

    s_idx = np.arange(128, dtype=np.int32)[:, None]
    c_idx = np.arange(1024, dtype=np.int32)[None, :]
    maskbig = (s_idx <= c_idx - 384).astype(ml_dtypes.bfloat16)

    in_maps = []
    for b in range(B):
        in_maps.append({
            "idxf": idx[b].astype(np.float32).reshape(1, T),
            "wte": wte,
            "wx": wx,
            "wy": wy,
            "enc": enc_s,
            "ro": readout,
            "cosT": cosT,
            "sinT": sinT,
            "maskbig": maskbig,
            "identm": np.eye(128, dtype=np.float32),
        })
    return in_maps


def kernel(idx, wte, encoder, decoder_x, decoder_y, readout):
    nc = get_nc()
    in_maps = make_host_inputs(idx, wte, encoder, decoder_x, decoder_y, readout)
    res = bass_utils.run_bass_kernel_spmd(nc, in_maps, core_ids=list(range(B)))
    out = np.stack([res.results[b]["logits"] for b in range(B)], axis=0)
    return out.astype(np.float32)
